# revision 1
# baseline (speedup 1.0000x reference)
"""Trainium2 Bass kernel for nn_CrossAttentionLayer (ragged cross-attention + MLP).

Sharding: 64 ragged segments -> 8 cores x 8 segments, each segment padded to
LMAX=512 slots. All activations are kept channel-major ("transposed", [chan, tok])
so every matmul contracts over the partition dim with no on-device transposes
except V (PE-transpose). Softmax runs in scoresT orientation [m_src, n_dst]:
the src-padding mask is a per-partition bias on the exp; the denominator is
computed with ones-lhsT matmuls that replicate each head's denominator across
its 32 partitions (so the normalization is a plain reciprocal + multiply).
Merge conv + BN are folded into the MLP weights on the host.
"""
import math
import sys
from contextlib import ExitStack

import numpy as np

try:
    import concourse.bass as bass
except ImportError:
    sys.path.insert(0, "/opt/trn_rl_repo")
    import concourse.bass as bass

import concourse.tile as tile
from concourse import bacc, mybir
from concourse.bass_utils import run_bass_kernel_spmd

F32 = mybir.dt.float32
F32R = mybir.dt.float32r

B = 64
LMAX = 512
H = 256          # h_dim
C = 128          # h_div
HEADS = 4
DH = 32
NCORES = 8
SEGS = 8         # segments per core
T = SEGS * LMAX  # padded tokens per core
NPB = 7          # per-partition bias columns: bq,bk,bv,b1a,b1b,b2a,b2b
MASK_NEG = -20000.0


def _r(ap):
    return ap if ap.dtype == F32R else ap.bitcast(F32R)


def host_prep(inputs):
    src_h = np.ascontiguousarray(np.asarray(inputs['src_h'], np.float32))
    dst_h = np.ascontiguousarray(np.asarray(inputs['dst_h'], np.float32))
    ns = np.asarray(inputs['src_num_verts']).astype(np.int64)
    nd = np.asarray(inputs['dst_num_verts']).astype(np.int64)
    soff = np.concatenate([[0], np.cumsum(ns)[:-1]])
    doff = np.concatenate([[0], np.cumsum(nd)[:-1]])

    perm = np.empty(C, np.int64)
    for chat in range(C):
        h, d = divmod(chat, DH)
        perm[chat] = d * HEADS + h
    s = 1.0 / math.sqrt(DH)

    f32 = lambda k: np.asarray(inputs[k], np.float32)
    Wq, bq = f32('Wq'), f32('bq')
    Wk, bk = f32('Wk'), f32('bk')
    Wv, bv = f32('Wv'), f32('bv')
    Wm, bm = f32('Wm'), f32('bm')
    W1, b1 = f32('W1'), f32('b1')
    W2, b2 = f32('W2'), f32('b2')
    g1, be1, rm1, rv1 = f32('g1'), f32('be1'), f32('rm1'), f32('rv1')
    g2, be2, rm2, rv2 = f32('g2'), f32('be2'), f32('rm2'), f32('rv2')

    WqT = np.ascontiguousarray((Wq[perm] * s).T)          # [256,128]
    bq_s = bq[perm] * s
    WkT = np.ascontiguousarray(Wk[perm].T)
    bk_r = bk[perm]
    WvT = np.ascontiguousarray(Wv[perm].T)
    bv_r = bv[perm]
    Wm_p = Wm[:, perm]
    a1 = g1 / np.sqrt(rv1 + 1e-5)
    W1_f = W1 * a1[:, None]
    b1_f = b1 * a1 + be1 - rm1 * a1
    a2 = g2 / np.sqrt(rv2 + 1e-5)
    W2_f = W2 * a2[:, None]
    b2_f = b2 * a2 + be2 - rm2 * a2
    W1m_p = W1_f[:, H:] @ Wm_p
    b1_p = b1_f + W1_f[:, H:] @ bm
    W1T = np.ascontiguousarray(np.concatenate([W1_f[:, :H], W1m_p], axis=1).T)  # [384,256]
    W2T = np.ascontiguousarray(W2_f.T)                    # [256,256]

    pbias = np.zeros((128, NPB), np.float32)
    pbias[:, 0] = bq_s
    pbias[:, 1] = bk_r
    pbias[:, 2] = bv_r
    pbias[:, 3] = b1_p[:128]
    pbias[:, 4] = b1_p[128:]
    pbias[:, 5] = b2_f[:128]
    pbias[:, 6] = b2_f[128:]

    cores = []
    for c in range(NCORES):
        dstT = np.zeros((H, T), np.float32)
        srcT = np.zeros((H, T), np.float32)
        maskb = np.full((128, SEGS * 4), MASK_NEG, np.float32)
        for si in range(SEGS):
            g = c * SEGS + si
            dstT[:, si * LMAX: si * LMAX + nd[g]] = dst_h[doff[g]:doff[g] + nd[g]].T
            srcT[:, si * LMAX: si * LMAX + ns[g]] = src_h[soff[g]:soff[g] + ns[g]].T
            for j in range(4):
                valid = max(0, min(128, int(ns[g]) - j * 128))
                maskb[:valid, si * 4 + j] = 0.0
        cores.append(dict(dstT=dstT, srcT=srcT, maskb=maskb))

    onespad = np.zeros((128, HEADS, C), np.float32)
    for h in range(HEADS):
        onespad[:, h, h * DH:(h + 1) * DH] = 1.0
    shared = dict(WqT=WqT, WkT=WkT, WvT=WvT, W1T=W1T, W2T=W2T, pbias=pbias,
                  onespad=onespad)
    meta = dict(nd=nd, doff=doff)
    return cores, shared, meta


def declare_tensors(nc):
    aps = {}
    aps['dstT'] = nc.dram_tensor("dstT", [H, T], F32R, kind="ExternalInput").ap()
    aps['srcT'] = nc.dram_tensor("srcT", [H, T], F32R, kind="ExternalInput").ap()
    aps['maskb'] = nc.dram_tensor("maskb", [128, SEGS * 4], F32, kind="ExternalInput").ap()
    aps['WqT'] = nc.dram_tensor("WqT", [H, C], F32R, kind="ExternalInput").ap()
    aps['WkT'] = nc.dram_tensor("WkT", [H, C], F32R, kind="ExternalInput").ap()
    aps['WvT'] = nc.dram_tensor("WvT", [H, C], F32R, kind="ExternalInput").ap()
    aps['W1T'] = nc.dram_tensor("W1T", [H + C, H], F32R, kind="ExternalInput").ap()
    aps['W2T'] = nc.dram_tensor("W2T", [H, H], F32R, kind="ExternalInput").ap()
    aps['pbias'] = nc.dram_tensor("pbias", [128, NPB], F32, kind="ExternalInput").ap()
    aps['onespad'] = nc.dram_tensor("onespad", [128, HEADS * C], F32R, kind="ExternalInput").ap()
    aps['eye'] = nc.dram_tensor("eye", [128, 128], F32R, kind="ExternalInput").ap()
    aps['vzero'] = nc.dram_tensor("vzero", [128, 4 * HEADS * C], F32R, kind="ExternalInput").ap()
    aps['outT'] = nc.dram_tensor("outT", [H, T], F32, kind="ExternalOutput").ap()
    return aps


def build_body(ctx: ExitStack, tc: tile.TileContext, aps, pfx=""):
    nc = tc.nc
    dstT_d, srcT_d, outT_d = aps['dstT'], aps['srcT'], aps['outT']

    wp = ctx.enter_context(tc.tile_pool(name=pfx + "wp", bufs=1))
    inp = ctx.enter_context(tc.tile_pool(name=pfx + "inp", bufs=1))
    qkv = ctx.enter_context(tc.tile_pool(name=pfx + "qkv", bufs=1))
    att = ctx.enter_context(tc.tile_pool(name=pfx + "att", bufs=1))
    mls = ctx.enter_context(tc.tile_pool(name=pfx + "mls", bufs=1))
    # PSUM pools: gp (proj+mlp, 2 banks) + sc (scores, 2x2 banks) + md (msg+den, 2 banks)
    gp = ctx.enter_context(tc.tile_pool(name=pfx + "gp", bufs=2, space="PSUM"))
    scp = ctx.enter_context(tc.tile_pool(name=pfx + "scp", bufs=2, space="PSUM"))
    mdp = ctx.enter_context(tc.tile_pool(name=pfx + "mdp", bufs=2, space="PSUM"))

    # --- weights ---
    wq = wp.tile([128, 2, C], F32R, tag="wq")
    wk = wp.tile([128, 2, C], F32R, tag="wk")
    wv = wp.tile([128, 2, C], F32R, tag="wv")
    w1 = wp.tile([128, 3, H], F32R, tag="w1")
    w2 = wp.tile([128, 2, H], F32R, tag="w2")
    pb = wp.tile([128, NPB], F32, tag="pb")
    maskb_t = wp.tile([128, SEGS * 4], F32, tag="maskb")
    onespad = wp.tile([128, HEADS, C], F32R, tag="onespad")
    eye = wp.tile([128, 128], F32R, tag="eye")
    for a in range(2):
        nc.sync.dma_start(out=wq[:, a, :], in_=aps['WqT'][a * 128:(a + 1) * 128, :])
        nc.sync.dma_start(out=wk[:, a, :], in_=aps['WkT'][a * 128:(a + 1) * 128, :])
        nc.sync.dma_start(out=wv[:, a, :], in_=aps['WvT'][a * 128:(a + 1) * 128, :])
        nc.sync.dma_start(out=w2[:, a, :], in_=aps['W2T'][a * 128:(a + 1) * 128, :])
    for a in range(3):
        nc.sync.dma_start(out=w1[:, a, :], in_=aps['W1T'][a * 128:(a + 1) * 128, :])
    nc.sync.dma_start(out=pb[:], in_=aps['pbias'][:])
    nc.sync.dma_start(out=maskb_t[:], in_=aps['maskb'][:])
    nc.sync.dma_start(out=onespad[:], in_=aps['onespad'].rearrange("p (h c) -> p h c", h=HEADS))
    nc.sync.dma_start(out=eye[:], in_=aps['eye'][:])

    # --- persistent V slots (zero-padded band layout), zero-filled once ---
    v_slots = []
    for i in range(3):
        vs = qkv.tile([128, 4, HEADS, C], F32R, tag=f"Vs{i}", name=f"Vs{i}")
        nc.sync.dma_start(out=vs[:], in_=aps['vzero'].rearrange("p (a h c) -> p a h c", a=4, h=HEADS))
        v_slots.append(vs)

    # --- persistent input tiles ---
    dst_t = [[None] * SEGS for _ in range(2)]
    src_t = [[None] * SEGS for _ in range(2)]
    for a in range(2):
        for s in range(SEGS):
            dt_ = inp.tile([128, LMAX], F32R, tag=f"dst{a}_{s}", name=f"dst{a}_{s}")
            nc.sync.dma_start(out=dt_[:], in_=dstT_d[a * 128:(a + 1) * 128, s * LMAX:(s + 1) * LMAX])
            dst_t[a][s] = dt_
            st_ = inp.tile([128, LMAX], F32R, tag=f"src{a}_{s}", name=f"src{a}_{s}")
            nc.sync.dma_start(out=st_[:], in_=srcT_d[a * 128:(a + 1) * 128, s * LMAX:(s + 1) * LMAX])
            src_t[a][s] = st_

    for s in range(SEGS):
        # ---------- projections for segment s ----------
        ps_q = gp.tile([128, LMAX], F32, tag="gp", name=f"psq{s}")
        for a in range(2):
            nc.tensor.matmul(ps_q[:], _r(wq[:, a, :]), _r(dst_t[a][s][:]),
                             start=(a == 0), stop=(a == 1))
        q_t = qkv.tile([128, LMAX], F32R, tag=f"q{s}", name=f"q{s}")
        nc.vector.tensor_scalar_add(q_t[:], ps_q[:], pb[:, 0:1])

        ps_k = gp.tile([128, LMAX], F32, tag="gp", name=f"psk{s}")
        for a in range(2):
            nc.tensor.matmul(ps_k[:], _r(wk[:, a, :]), _r(src_t[a][s][:]),
                             start=(a == 0), stop=(a == 1))
        k_t = qkv.tile([128, LMAX], F32R, tag=f"k{s}", name=f"k{s}")
        nc.vector.tensor_scalar_add(k_t[:], ps_k[:], pb[:, 1:2])

        ps_v = gp.tile([128, LMAX], F32, tag="gp", name=f"psv{s}")
        for a in range(2):
            nc.tensor.matmul(ps_v[:], _r(wv[:, a, :]), _r(src_t[a][s][:]),
                             start=(a == 0), stop=(a == 1))
        vT_t = qkv.tile([128, LMAX], F32R, tag="vt", name=f"vt{s}", bufs=2)
        nc.vector.tensor_scalar_add(vT_t[:], ps_v[:], pb[:, 2:3])

        # transpose vT [chan,tok] -> V natural [tok, chan] (4 chunks of 128 toks)
        ps_tr = gp.tile([128, 4, 128], F32R, tag="gp", name=f"pstr{s}")
        for j in range(4):
            nc.tensor.transpose(ps_tr[:, j, :], vT_t[:, j * 128:(j + 1) * 128], eye[:])
        v_sb = v_slots[s % 3]
        # scatter each transposed [128,128] j-block into its 4 per-head bands
        vdst = bass.AP(tensor=v_sb.tensor, offset=v_sb.offset,
                       ap=[v_sb.ap[0]] + [[HEADS * C, 4], [C + DH, HEADS], [1, DH]])
        vsrc = bass.AP(tensor=ps_tr.tensor, offset=ps_tr.offset,
                       ap=[ps_tr.ap[0]] + [[128, 4], [DH, HEADS], [1, DH]])
        nc.vector.tensor_copy(vdst, vsrc)

        # ---------- attention for segment s ----------
        ps_msg = mdp.tile([128, LMAX], F32, tag="md", name=f"psmsg{s}")
        ps_den = mdp.tile([128, LMAX], F32, tag="md", name=f"psden{s}")
        for j in range(4):
            for hp in range(2):
                ps_sc = scp.tile([128, 2, LMAX], F32, tag="sc", name=f"pssc{s}_{j}_{hp}")
                for hh in range(2):
                    h = 2 * hp + hh
                    nc.tensor.matmul(
                        ps_sc[:, hh, :],
                        _r(k_t[32 * h:32 * h + 32, j * 128:(j + 1) * 128]),
                        _r(q_t[32 * h:32 * h + 32, :]),
                        start=True, stop=True, tile_position=(32 * h, 0))
                e_t = att.tile([128, 2, LMAX], F32R, tag="E", name=f"E{s}_{j}_{hp}", bufs=5)
                nc.scalar.activation(e_t[:], ps_sc[:],
                                     mybir.ActivationFunctionType.Exp,
                                     bias=maskb_t[:, s * 4 + j: s * 4 + j + 1])
                for hh in range(2):
                    h = 2 * hp + hh
                    first = (j == 0 and h == 0)
                    last = (j == 3 and h == 3)
                    nc.tensor.matmul(
                        ps_msg[:, :],
                        v_sb[:, j, h, :],
                        e_t[:, hh, :],
                        start=first, stop=last)
                    # den replicated over the head's 32 partitions: this IS the
                    # broadcast needed for the normalization divide below
                    nc.tensor.matmul(
                        ps_den[:, :],
                        onespad[:, h, :],
                        e_t[:, hh, :],
                        start=first, stop=last)
        r_sb = att.tile([128, LMAX], F32, tag="rsb", name=f"rsb{s}", bufs=2)
        nc.vector.reciprocal(r_sb[:], ps_den[:])
        msgn = att.tile([128, LMAX], F32R, tag="msgn", name=f"msgn{s}", bufs=3)
        nc.vector.tensor_mul(msgn[:], ps_msg[:], r_sb[:])

        # ---------- MLP for segment s (merge folded into W1) ----------
        y1 = [None, None]
        for o in range(2):
            ps_y = gp.tile([128, LMAX], F32, tag="gp", name=f"psy{s}_{o}")
            rhs_list = [dst_t[0][s], dst_t[1][s], msgn]
            for kk in range(3):
                nc.tensor.matmul(ps_y[:], _r(w1[:, kk, o * 128:(o + 1) * 128]),
                                 _r(rhs_list[kk][:]), start=(kk == 0), stop=(kk == 2))
            y1_t = mls.tile([128, LMAX], F32R, tag=f"y1_{o}", name=f"y1_{s}_{o}", bufs=2)
            nc.vector.tensor_scalar(y1_t[:], ps_y[:], pb[:, 3 + o:4 + o], 0.0,
                                    op0=mybir.AluOpType.add, op1=mybir.AluOpType.max)
            y1[o] = y1_t
        for o in range(2):
            ps_z = gp.tile([128, LMAX], F32, tag="gp", name=f"psz{s}_{o}")
            for kk in range(2):
                nc.tensor.matmul(ps_z[:], w2[:, kk, o * 128:(o + 1) * 128],
                                 y1[kk][:], start=(kk == 0), stop=(kk == 1))
            zt = mls.tile([128, LMAX], F32, tag=f"zt_{o}", name=f"zt_{s}_{o}", bufs=2)
            nc.vector.tensor_scalar_add(zt[:], ps_z[:], pb[:, 5 + o:6 + o])
            out_sb = mls.tile([128, LMAX], F32, tag=f"out_{o}", name=f"out_{s}_{o}", bufs=2)
            nc.gpsimd.tensor_add(out_sb[:], zt[:], dst_t[o][s][:].bitcast(F32))
            nc.sync.dma_start(out=outT_d[o * 128:(o + 1) * 128, s * LMAX:(s + 1) * LMAX],
                              in_=out_sb[:])


def build_nc(reps=1):
    nc = bacc.Bacc("TRN2", target_bir_lowering=False, debug=False,
                   enable_asserts=True, num_devices=NCORES)
    aps = declare_tensors(nc)
    with tile.TileContext(nc) as tc:
        for rep in range(reps):
            with ExitStack() as ctx:
                build_body(ctx, tc, aps, pfx=f"r{rep}" if rep else "")
    nc.compile()
    return nc


def in_map(core, shared):
    m = dict(dstT=core['dstT'], srcT=core['srcT'], maskb=core['maskb'])
    m['onespad'] = shared['onespad'].reshape(128, HEADS * C)
    m['vzero'] = np.zeros((128, 4 * HEADS * C), np.float32)
    m['eye'] = np.eye(128, dtype=np.float32)
    m.update({k: shared[k] for k in ('WqT', 'WkT', 'WvT', 'W1T', 'W2T', 'pbias')})
    return m


def assemble(outTs, meta):
    nd = meta['nd']
    doff = meta['doff']
    out = np.empty((int(nd.sum()), H), np.float32)
    for c in range(NCORES):
        for si in range(SEGS):
            g = c * SEGS + si
            out[doff[g]:doff[g] + nd[g]] = outTs[c][:, si * LMAX: si * LMAX + nd[g]].T
    return out


def kernel(**inputs):
    cores, shared, meta = host_prep(inputs)
    nc = build_nc()
    in_maps = [in_map(cores[c], shared) for c in range(NCORES)]
    res = run_bass_kernel_spmd(nc, in_maps, core_ids=list(range(NCORES)))
    outTs = [res.results[c]["outT"] for c in range(NCORES)]
    return assemble(outTs, meta)



# revision 7
# speedup vs baseline: 1.4483x; 1.4483x over previous
"""Trainium2 Bass kernel for nn_CrossAttentionLayer (ragged cross-attention + MLP).

Sharding: 64 ragged segments -> 8 cores x 8 slots. Segments are sorted by
(src-chunk count, dst count) and dealt so the 8 segments sharing a slot have
similar sizes; each slot is trimmed to the max dst count (ND) / src count
(NS, rounded to 128) over its 8 segments, so all cores run one SPMD program.

All matmul operands are bf16 (fp32 PSUM accumulation). Activations stay
channel-major [chan, tok]; softmax runs in scoresT orientation [src, dst]
with the src-padding mask applied as a per-partition bias on the exp. The
denominator is computed with banded-ones matmuls replicating each head's
denominator across its 32 partitions; normalization is reciprocal_approx_fast
+ multiply. V is produced directly in natural [tok, chan] orientation (src
chunks as stationary), with its bias folded into the normalization:
msgn = msg * recip(den) + bv. The dst residual is added via an identity
matmul accumulated into the MLP2 PSUM group. Merge conv + BN are folded into
the MLP weights on the host.
"""
import math
import sys
from contextlib import ExitStack

import numpy as np
import ml_dtypes

try:
    import concourse.bass as bass
except ImportError:
    sys.path.insert(0, "/opt/trn_rl_repo")
    import concourse.bass as bass

import concourse.tile as tile
from concourse import bacc, mybir
from concourse.bass_utils import run_bass_kernel_spmd

F32 = mybir.dt.float32
F32R = mybir.dt.float32r
BF16 = mybir.dt.bfloat16
BF = ml_dtypes.bfloat16

H = 256          # h_dim
C = 128          # h_div
HEADS = 4
DH = 32
NCORES = 8
NPB = 7          # per-partition bias columns: bq,bk,b1a,b1b,b2a,b2b,bv
MASK_NEG = -20000.0

# Filled by host_prep; read by build_nc/in_map (same process).
_PLAN = {}


def _make_plan(nd, ns):
    """Assign 64 segments to (core, slot); compute per-slot widths."""
    B = len(nd)
    slots = B // NCORES
    chunks_of = np.ceil(ns / 128).astype(int)
    # sort by (chunks desc, nd desc); deal consecutive groups of 8 per slot
    order = np.lexsort((-nd, -chunks_of))
    assign = np.empty((NCORES, slots), dtype=int)   # (core, slot) -> segment
    ND = np.empty(slots, dtype=int)
    NS = np.empty(slots, dtype=int)
    for j in range(slots):
        grp = order[j * NCORES:(j + 1) * NCORES]
        for c in range(NCORES):
            assign[c, j] = grp[c]
        ND[j] = int(-(-max(nd[g] for g in grp) // 4) * 4)       # mult of 4
        NS[j] = int(-(-max(ns[g] for g in grp) // 128) * 128)   # mult of 128
    CH = (NS // 128).astype(int)
    return dict(slots=slots, assign=assign, ND=ND, NS=NS, CH=CH,
                doff=np.concatenate([[0], np.cumsum(ND)[:-1]]),
                soff=np.concatenate([[0], np.cumsum(NS)[:-1]]),
                moff=np.concatenate([[0], np.cumsum(CH)[:-1]]),
                WD=int(ND.sum()), WS=int(NS.sum()), WM=int(CH.sum()))


def host_prep(inputs):
    src_h = np.asarray(inputs['src_h'], np.float32)
    dst_h = np.asarray(inputs['dst_h'], np.float32)
    ns = np.asarray(inputs['src_num_verts']).astype(np.int64)
    nd = np.asarray(inputs['dst_num_verts']).astype(np.int64)
    soff_g = np.concatenate([[0], np.cumsum(ns)[:-1]])
    doff_g = np.concatenate([[0], np.cumsum(nd)[:-1]])

    plan = _make_plan(nd, ns)
    global _PLAN
    _PLAN = plan
    slots = plan['slots']

    perm = np.empty(C, np.int64)
    for chat in range(C):
        h, d = divmod(chat, DH)
        perm[chat] = d * HEADS + h
    s = 1.0 / math.sqrt(DH)

    f32 = lambda k: np.asarray(inputs[k], np.float32)
    Wq, bq = f32('Wq'), f32('bq')
    Wk, bk = f32('Wk'), f32('bk')
    Wv, bv = f32('Wv'), f32('bv')
    Wm, bm = f32('Wm'), f32('bm')
    W1, b1 = f32('W1'), f32('b1')
    W2, b2 = f32('W2'), f32('b2')
    g1, be1, rm1, rv1 = f32('g1'), f32('be1'), f32('rm1'), f32('rv1')
    g2, be2, rm2, rv2 = f32('g2'), f32('be2'), f32('rm2'), f32('rv2')

    WqT = np.ascontiguousarray((Wq[perm] * s).T)          # [256,128]
    bq_s = bq[perm] * s
    WkT = np.ascontiguousarray(Wk[perm].T)
    bk_r = bk[perm]
    WvT = np.ascontiguousarray(Wv[perm].T)                # [256,128] moving for V-direct
    bv_r = bv[perm]
    Wm_p = Wm[:, perm]
    a1 = g1 / np.sqrt(rv1 + 1e-5)
    W1_f = W1 * a1[:, None]
    b1_f = b1 * a1 + be1 - rm1 * a1
    a2 = g2 / np.sqrt(rv2 + 1e-5)
    W2_f = W2 * a2[:, None]
    b2_f = b2 * a2 + be2 - rm2 * a2
    W1m_p = W1_f[:, H:] @ Wm_p
    # V bias folded all the way into the MLP1 bias: msg enters MLP1 as
    # msg/den (no bias), and W1m_p @ bv_perm is a constant.
    b1_p = b1_f + W1_f[:, H:] @ bm + W1m_p @ bv_r
    W1T = np.ascontiguousarray(np.concatenate([W1_f[:, :H], W1m_p], axis=1).T)  # [384,256]
    W2T = np.ascontiguousarray(W2_f.T)                    # [256,256]

    pbias = np.zeros((128, NPB), np.float32)
    pbias[:, 0] = bq_s
    pbias[:, 1] = bk_r
    pbias[:, 2] = b1_p[:128]
    pbias[:, 3] = b1_p[128:]
    pbias[:, 4] = b2_f[:128]
    pbias[:, 5] = b2_f[128:]

    ND, NS, CH = plan['ND'], plan['NS'], plan['CH']
    doff, soff, moff = plan['doff'], plan['soff'], plan['moff']
    cores = []
    for c in range(NCORES):
        dstT = np.zeros((H, plan['WD']), BF)
        srcT = np.zeros((H, plan['WS']), BF)
        maskb = np.full((128, plan['WM']), MASK_NEG, np.float32)
        for j in range(slots):
            g = plan['assign'][c, j]
            dstT[:, doff[j]:doff[j] + nd[g]] = dst_h[doff_g[g]:doff_g[g] + nd[g]].T.astype(BF)
            srcT[:, soff[j]:soff[j] + ns[g]] = src_h[soff_g[g]:soff_g[g] + ns[g]].T.astype(BF)
            for jj in range(CH[j]):
                valid = max(0, min(128, int(ns[g]) - jj * 128))
                maskb[:valid, moff[j] + jj] = 0.0
        cores.append(dict(dstT=dstT, srcT=srcT, maskb=maskb))

    onespad = np.zeros((128, HEADS, C), BF)
    for h in range(HEADS):
        onespad[:, h, h * DH:(h + 1) * DH] = 1.0
    shared = dict(
        WqT=WqT.astype(BF), WkT=WkT.astype(BF), WvT=WvT.astype(BF),
        W1T=W1T.astype(BF), W2T=W2T.astype(BF), pbias=pbias,
        onespad=onespad.reshape(128, HEADS * C),
        eye=np.eye(128, dtype=BF),
    )
    meta = dict(nd=nd, doff_g=doff_g, plan=plan)
    return cores, shared, meta


def declare_tensors(nc, plan):
    aps = {}
    aps['dstT'] = nc.dram_tensor("dstT", [H, plan['WD']], BF16, kind="ExternalInput").ap()
    aps['srcT'] = nc.dram_tensor("srcT", [H, plan['WS']], BF16, kind="ExternalInput").ap()
    aps['maskb'] = nc.dram_tensor("maskb", [128, plan['WM']], F32, kind="ExternalInput").ap()
    aps['WqT'] = nc.dram_tensor("WqT", [H, C], BF16, kind="ExternalInput").ap()
    aps['WkT'] = nc.dram_tensor("WkT", [H, C], BF16, kind="ExternalInput").ap()
    aps['WvT'] = nc.dram_tensor("WvT", [H, C], BF16, kind="ExternalInput").ap()
    aps['W1T'] = nc.dram_tensor("W1T", [H + C, H], BF16, kind="ExternalInput").ap()
    aps['W2T'] = nc.dram_tensor("W2T", [H, H], BF16, kind="ExternalInput").ap()
    aps['pbias'] = nc.dram_tensor("pbias", [128, NPB], F32, kind="ExternalInput").ap()
    aps['onespad'] = nc.dram_tensor("onespad", [128, HEADS * C], BF16, kind="ExternalInput").ap()
    aps['eye'] = nc.dram_tensor("eye", [128, 128], BF16, kind="ExternalInput").ap()
    aps['outT'] = nc.dram_tensor("outT", [H, plan['WD']], BF16, kind="ExternalOutput").ap()
    return aps


def build_body(ctx: ExitStack, tc: tile.TileContext, aps, plan):
    nc = tc.nc
    slots = plan['slots']
    ND, NS, CH = plan['ND'], plan['NS'], plan['CH']
    doff, soff, moff = plan['doff'], plan['soff'], plan['moff']
    NDmax = int(ND.max())
    NSmax = int(NS.max())
    CHmax = int(CH.max())

    wp = ctx.enter_context(tc.tile_pool(name="wp", bufs=1))
    inp = ctx.enter_context(tc.tile_pool(name="inp", bufs=3))
    act = ctx.enter_context(tc.tile_pool(name="act", bufs=1))
    # PSUM: gpp(proj q/k + v-direct) 2 + sc 2 + msg 1 + den 1 + mlp 2 = 8 banks
    gpp = ctx.enter_context(tc.tile_pool(name="gpp", bufs=2, space="PSUM"))
    scp = ctx.enter_context(tc.tile_pool(name="scp", bufs=1, space="PSUM"))
    mdp = ctx.enter_context(tc.tile_pool(name="mdp", bufs=1, space="PSUM"))
    mlp = ctx.enter_context(tc.tile_pool(name="mlp", bufs=2, space="PSUM"))

    # --- weights (persistent) ---
    wq = wp.tile([128, 2, C], BF16, tag="wq")
    wk = wp.tile([128, 2, C], BF16, tag="wk")
    wv = wp.tile([128, 2, C], BF16, tag="wv")
    w1 = wp.tile([128, 3, H], BF16, tag="w1")
    w2 = wp.tile([128, 2, H], BF16, tag="w2")
    pb = wp.tile([128, NPB], F32, tag="pb")
    maskb_t = wp.tile([128, plan['WM']], F32, tag="maskb")
    onespad = wp.tile([128, HEADS, C], BF16, tag="onespad")
    eye = wp.tile([128, 128], BF16, tag="eye")
    for a in range(2):
        nc.sync.dma_start(out=wq[:, a, :], in_=aps['WqT'][a * 128:(a + 1) * 128, :])
        nc.sync.dma_start(out=wk[:, a, :], in_=aps['WkT'][a * 128:(a + 1) * 128, :])
        nc.sync.dma_start(out=wv[:, a, :], in_=aps['WvT'][a * 128:(a + 1) * 128, :])
        nc.sync.dma_start(out=w2[:, a, :], in_=aps['W2T'][a * 128:(a + 1) * 128, :])
    for a in range(3):
        nc.sync.dma_start(out=w1[:, a, :], in_=aps['W1T'][a * 128:(a + 1) * 128, :])
    nc.sync.dma_start(out=pb[:], in_=aps['pbias'][:])
    nc.sync.dma_start(out=maskb_t[:], in_=aps['maskb'][:])
    nc.sync.dma_start(out=onespad[:], in_=aps['onespad'].rearrange("p (h c) -> p h c", h=HEADS))
    nc.sync.dma_start(out=eye[:], in_=aps['eye'][:])

    # --- persistent V slots (zero-padded band layout), zero-filled once ---
    v_slots = []
    for i in range(3):
        vs = act.tile([128, CHmax, HEADS, C], BF16, tag=f"Vs{i}", name=f"Vs{i}")
        nc.gpsimd.memset(vs[:], 0.0)
        v_slots.append(vs)

    # --- input tiles: 3-buffer rotation, 2-slot DMA lookahead ---
    dst_t = [None] * slots
    src_t = [None] * slots

    def load_slot(s):
        dt_ = [inp.tile([128, ND[s]], BF16, tag=f"dst{a}", name=f"dst{a}_{s}")
               for a in range(2)]
        st_ = [inp.tile([128, NS[s]], BF16, tag=f"src{a}", name=f"src{a}_{s}")
               for a in range(2)]
        for a in range(2):
            nc.sync.dma_start(out=dt_[a][:],
                              in_=aps['dstT'][a * 128:(a + 1) * 128, doff[s]:doff[s] + ND[s]])
            nc.sync.dma_start(out=st_[a][:],
                              in_=aps['srcT'][a * 128:(a + 1) * 128, soff[s]:soff[s] + NS[s]])
        dst_t[s] = dt_
        src_t[s] = st_

    load_slot(0)
    if slots > 1:
        load_slot(1)

    for s in range(slots):
        nd_, ns_, ch = int(ND[s]), int(NS[s]), int(CH[s])
        if s + 2 < slots:
            load_slot(s + 2)
        dt_, st_ = dst_t[s], src_t[s]

        # ---------- projections ----------
        ps_q = gpp.tile([128, NDmax], F32, tag="gpp", name=f"psq{s}")
        for a in range(2):
            nc.tensor.matmul(ps_q[:, :nd_], wq[:, a, :], dt_[a][:],
                             start=(a == 0), stop=(a == 1))
        q_t = act.tile([128, NDmax], BF16, tag="q", name=f"q{s}", bufs=2)
        nc.vector.tensor_scalar_add(q_t[:, :nd_], ps_q[:, :nd_], pb[:, 0:1])

        ps_k = gpp.tile([128, NSmax], F32, tag="gpp", name=f"psk{s}")
        for a in range(2):
            nc.tensor.matmul(ps_k[:, :ns_], wk[:, a, :], st_[a][:],
                             start=(a == 0), stop=(a == 1))
        k_t = act.tile([128, NSmax], BF16, tag="k", name=f"k{s}", bufs=2)
        nc.vector.tensor_scalar_add(k_t[:, :ns_], ps_k[:, :ns_], pb[:, 1:2])

        # ---------- V direct (natural [tok, chan]); bias folded into msgn ----------
        ps_vd = gpp.tile([128, CHmax, 128], F32, tag="gpp", name=f"psvd{s}")
        for j in range(ch):
            for a in range(2):
                nc.tensor.matmul(ps_vd[:, j, :],
                                 st_[a][:, j * 128:(j + 1) * 128], wv[:, a, :],
                                 start=(a == 0), stop=(a == 1))
        v_sb = v_slots[s % 3]
        vdst = bass.AP(tensor=v_sb.tensor, offset=v_sb.offset,
                       ap=[v_sb.ap[0]] + [[HEADS * C, ch], [C + DH, HEADS], [1, DH]])
        vsrc = bass.AP(tensor=ps_vd.tensor, offset=ps_vd.offset,
                       ap=[ps_vd.ap[0]] + [[128, ch], [DH, HEADS], [1, DH]])
        nc.vector.tensor_copy(vdst, vsrc)

        # ---------- attention ----------
        ps_msg = mdp.tile([128, NDmax], F32, tag="msg", name=f"psmsg{s}")
        ps_den = mdp.tile([128, NDmax], F32, tag="den", name=f"psden{s}")
        for j in range(ch):
            for hp in range(2):
                ps_sc = scp.tile([128, 2, 512], F32, tag="sc", name=f"pssc{s}_{j}_{hp}")
                for hh in range(2):
                    h = 2 * hp + hh
                    nc.tensor.matmul(
                        ps_sc[:, hh, :nd_],
                        k_t[32 * h:32 * h + 32, j * 128:(j + 1) * 128],
                        q_t[32 * h:32 * h + 32, :nd_],
                        start=True, stop=True, tile_position=(32 * h, 0))
                e_t = act.tile([128, 2, NDmax], BF16, tag="E", name=f"E{s}_{j}_{hp}", bufs=6)
                nc.scalar.activation(e_t[:, :, :nd_], ps_sc[:, :, :nd_],
                                     mybir.ActivationFunctionType.Exp,
                                     bias=maskb_t[:, moff[s] + j: moff[s] + j + 1])
                for hh in range(2):
                    h = 2 * hp + hh
                    first = (j == 0 and h == 0)
                    last = (j == ch - 1 and h == 3)
                    nc.tensor.matmul(
                        ps_msg[:, :nd_],
                        v_sb[:, j, h, :],
                        e_t[:, hh, :nd_],
                        start=first, stop=last)
                    nc.tensor.matmul(
                        ps_den[:, :nd_],
                        onespad[:, h, :],
                        e_t[:, hh, :nd_],
                        start=first, stop=last)
        r_sb = act.tile([128, NDmax], F32, tag="rsb", name=f"rsb{s}", bufs=2)
        nc.vector.reciprocal_approx_fast(r_sb[:, :nd_], ps_den[:, :nd_])
        msgn = act.tile([128, NDmax], BF16, tag="msgn", name=f"msgn{s}", bufs=2)
        nc.vector.tensor_mul(msgn[:, :nd_], ps_msg[:, :nd_], r_sb[:, :nd_])

        # ---------- MLP (merge folded into W1; residual via eye matmul) ----------
        y1 = [None, None]
        for o in range(2):
            ps_y = mlp.tile([128, NDmax], F32, tag="mlp", name=f"psy{s}_{o}")
            rhs_list = [dt_[0][:], dt_[1][:], msgn[:, :nd_]]
            for kk in range(3):
                nc.tensor.matmul(ps_y[:, :nd_], w1[:, kk, o * 128:(o + 1) * 128],
                                 rhs_list[kk], start=(kk == 0), stop=(kk == 2))
            y1_t = act.tile([128, NDmax], BF16, tag=f"y1_{o}", name=f"y1_{s}_{o}", bufs=2)
            nc.vector.tensor_scalar(y1_t[:, :nd_], ps_y[:, :nd_], pb[:, 2 + o:3 + o], 0.0,
                                    op0=mybir.AluOpType.add, op1=mybir.AluOpType.max)
            y1[o] = y1_t
        for o in range(2):
            ps_z = mlp.tile([128, NDmax], F32, tag="mlp", name=f"psz{s}_{o}")
            for kk in range(2):
                nc.tensor.matmul(ps_z[:, :nd_], w2[:, kk, o * 128:(o + 1) * 128],
                                 y1[kk][:, :nd_], start=(kk == 0), stop=False)
            nc.tensor.matmul(ps_z[:, :nd_], eye[:], dt_[o][:],
                             start=False, stop=True)
            out_sb = act.tile([128, NDmax], BF16, tag=f"out_{o}", name=f"out_{s}_{o}", bufs=2)
            nc.vector.tensor_scalar_add(out_sb[:, :nd_], ps_z[:, :nd_], pb[:, 4 + o:5 + o])
            nc.gpsimd.dma_start(out=aps['outT'][o * 128:(o + 1) * 128, doff[s]:doff[s] + nd_],
                                in_=out_sb[:, :nd_])


def build_nc(plan=None):
    if plan is None:
        plan = _PLAN
    nc = bacc.Bacc("TRN2", target_bir_lowering=False, debug=False,
                   enable_asserts=True, num_devices=NCORES)
    aps = declare_tensors(nc, plan)
    with tile.TileContext(nc) as tc:
        with ExitStack() as ctx:
            build_body(ctx, tc, aps, plan)
    nc.compile()
    return nc


def in_map(core, shared):
    m = dict(dstT=core['dstT'], srcT=core['srcT'], maskb=core['maskb'])
    m.update({k: shared[k] for k in ('WqT', 'WkT', 'WvT', 'W1T', 'W2T',
                                     'pbias', 'onespad', 'eye')})
    return m


def assemble(outTs, meta):
    nd = meta['nd']
    doff_g = meta['doff_g']
    plan = meta['plan']
    out = np.empty((int(nd.sum()), H), np.float32)
    for c in range(NCORES):
        for j in range(plan['slots']):
            g = plan['assign'][c, j]
            sl = outTs[c][:, plan['doff'][j]: plan['doff'][j] + nd[g]]
            out[doff_g[g]:doff_g[g] + nd[g]] = sl.T.astype(np.float32)
    return out


def kernel(**inputs):
    cores, shared, meta = host_prep(inputs)
    nc = build_nc(meta['plan'])
    in_maps = [in_map(cores[c], shared) for c in range(NCORES)]
    res = run_bass_kernel_spmd(nc, in_maps, core_ids=list(range(NCORES)))
    outTs = [np.asarray(res.results[c]["outT"]) for c in range(NCORES)]
    return assemble(outTs, meta)


# revision 11
# speedup vs baseline: 2.1472x; 1.4825x over previous
"""Trainium2 Bass kernel for nn_CrossAttentionLayer (ragged cross-attention + MLP).

Sharding: 64 ragged segments -> 8 cores x 8 slots. Segments are sorted by
(src-chunk count, dst count) and dealt so the 8 segments sharing a slot have
similar sizes; each slot is trimmed to the max dst count (ND) / src count
(NS, rounded to 128) over its 8 segments, so all cores run one SPMD program.

All matmul operands are bf16 (fp32 PSUM accumulation). Activations stay
channel-major [chan, tok]; softmax runs in scoresT orientation [src, dst]
with the src-padding mask applied as a per-partition bias on the exp. The
denominator is computed with banded-ones matmuls replicating each head's
denominator across its 32 partitions; normalization is reciprocal_approx_fast
+ multiply. V is produced directly in natural [tok, chan] orientation (src
chunks as stationary), with its bias folded into the normalization:
msgn = msg * recip(den) + bv. The dst residual is added via an identity
matmul accumulated into the MLP2 PSUM group. Merge conv + BN are folded into
the MLP weights on the host.
"""
import math
import sys
from contextlib import ExitStack

import numpy as np
import ml_dtypes

try:
    import concourse.bass as bass
except ImportError:
    sys.path.insert(0, "/opt/trn_rl_repo")
    import concourse.bass as bass

import concourse.tile as tile
from concourse import bacc, mybir
from concourse.bass_utils import run_bass_kernel_spmd

F32 = mybir.dt.float32
F32R = mybir.dt.float32r
BF16 = mybir.dt.bfloat16
BF = ml_dtypes.bfloat16

H = 256          # h_dim
C = 128          # h_div
HEADS = 4
DH = 32
NCORES = 8
NPB = 7          # per-partition bias columns: bq,bk,b1a,b1b,b2a,b2b,bv
MASK_NEG = -20000.0

# Filled by host_prep; read by build_nc/in_map (same process).
_PLAN = {}


def _make_plan(nd, ns):
    """Assign 64 segments to (core, slot); compute per-slot widths."""
    B = len(nd)
    slots = B // NCORES
    chunks_of = np.ceil(ns / 128).astype(int)
    # sort by (chunks desc, nd desc); deal consecutive groups of 8 per slot
    order = np.lexsort((-nd, -chunks_of))
    assign = np.empty((NCORES, slots), dtype=int)   # (core, slot) -> segment
    ND = np.empty(slots, dtype=int)
    NS = np.empty(slots, dtype=int)
    for j in range(slots):
        grp = order[j * NCORES:(j + 1) * NCORES]
        for c in range(NCORES):
            assign[c, j] = grp[c]
        ND[j] = int(-(-max(nd[g] for g in grp) // 4) * 4)       # mult of 4
        NS[j] = int(-(-max(ns[g] for g in grp) // 128) * 128)   # mult of 128
    CH = (NS // 128).astype(int)
    return dict(slots=slots, assign=assign, ND=ND, NS=NS, CH=CH,
                doff=np.concatenate([[0], np.cumsum(ND)[:-1]]),
                soff=np.concatenate([[0], np.cumsum(NS)[:-1]]),
                moff=np.concatenate([[0], np.cumsum(CH)[:-1]]),
                WD=int(ND.sum()), WS=int(NS.sum()), WM=int(CH.sum()))


def host_prep(inputs):
    src_h = np.asarray(inputs['src_h'], np.float32)
    dst_h = np.asarray(inputs['dst_h'], np.float32)
    ns = np.asarray(inputs['src_num_verts']).astype(np.int64)
    nd = np.asarray(inputs['dst_num_verts']).astype(np.int64)
    soff_g = np.concatenate([[0], np.cumsum(ns)[:-1]])
    doff_g = np.concatenate([[0], np.cumsum(nd)[:-1]])

    plan = _make_plan(nd, ns)
    global _PLAN
    _PLAN = plan
    slots = plan['slots']

    perm = np.empty(C, np.int64)
    for chat in range(C):
        h, d = divmod(chat, DH)
        perm[chat] = d * HEADS + h
    s = 1.0 / math.sqrt(DH)

    f32 = lambda k: np.asarray(inputs[k], np.float32)
    Wq, bq = f32('Wq'), f32('bq')
    Wk, bk = f32('Wk'), f32('bk')
    Wv, bv = f32('Wv'), f32('bv')
    Wm, bm = f32('Wm'), f32('bm')
    W1, b1 = f32('W1'), f32('b1')
    W2, b2 = f32('W2'), f32('b2')
    g1, be1, rm1, rv1 = f32('g1'), f32('be1'), f32('rm1'), f32('rv1')
    g2, be2, rm2, rv2 = f32('g2'), f32('be2'), f32('rm2'), f32('rv2')

    WqT = np.ascontiguousarray((Wq[perm] * s).T)          # [256,128]
    bq_s = bq[perm] * s
    WkT = np.ascontiguousarray(Wk[perm].T)
    bk_r = bk[perm]
    WvT = np.ascontiguousarray(Wv[perm].T)                # [256,128] moving for V-direct
    bv_r = bv[perm]
    Wm_p = Wm[:, perm]
    a1 = g1 / np.sqrt(rv1 + 1e-5)
    W1_f = W1 * a1[:, None]
    b1_f = b1 * a1 + be1 - rm1 * a1
    a2 = g2 / np.sqrt(rv2 + 1e-5)
    W2_f = W2 * a2[:, None]
    b2_f = b2 * a2 + be2 - rm2 * a2
    W1m_p = W1_f[:, H:] @ Wm_p
    # V bias folded all the way into the MLP1 bias: msg enters MLP1 as
    # msg/den (no bias), and W1m_p @ bv_perm is a constant.
    b1_p = b1_f + W1_f[:, H:] @ bm + W1m_p @ bv_r
    W1T = np.ascontiguousarray(np.concatenate([W1_f[:, :H], W1m_p], axis=1).T)  # [384,256]
    W2T = np.ascontiguousarray(W2_f.T)                    # [256,256]

    pbias = np.zeros((128, NPB), np.float32)
    pbias[:, 0] = bq_s
    pbias[:, 1] = bk_r
    pbias[:, 2] = b1_p[:128]
    pbias[:, 3] = b1_p[128:]
    pbias[:, 4] = b2_f[:128]
    pbias[:, 5] = b2_f[128:]

    ND, NS, CH = plan['ND'], plan['NS'], plan['CH']
    doff, soff, moff = plan['doff'], plan['soff'], plan['moff']
    cores = []
    for c in range(NCORES):
        dstT = np.zeros((H, plan['WD']), BF)
        srcT = np.zeros((H, plan['WS']), BF)
        maskb = np.full((128, plan['WM']), MASK_NEG, np.float32)
        for j in range(slots):
            g = plan['assign'][c, j]
            dstT[:, doff[j]:doff[j] + nd[g]] = dst_h[doff_g[g]:doff_g[g] + nd[g]].T.astype(BF)
            srcT[:, soff[j]:soff[j] + ns[g]] = src_h[soff_g[g]:soff_g[g] + ns[g]].T.astype(BF)
            for jj in range(CH[j]):
                valid = max(0, min(128, int(ns[g]) - jj * 128))
                maskb[:valid, moff[j] + jj] = 0.0
        cores.append(dict(dstT=dstT, srcT=srcT, maskb=maskb))

    onespad = np.zeros((128, HEADS, C), BF)
    for h in range(HEADS):
        onespad[:, h, h * DH:(h + 1) * DH] = 1.0
    shared = dict(
        WqT=WqT.astype(BF), WkT=WkT.astype(BF), WvT=WvT.astype(BF),
        W1T=W1T.astype(BF), W2T=W2T.astype(BF), pbias=pbias,
        onespad=onespad.reshape(128, HEADS * C),
        eye=np.eye(128, dtype=BF),
    )
    meta = dict(nd=nd, doff_g=doff_g, plan=plan)
    return cores, shared, meta


def declare_tensors(nc, plan):
    aps = {}
    aps['dstT'] = nc.dram_tensor("dstT", [H, plan['WD']], BF16, kind="ExternalInput").ap()
    aps['srcT'] = nc.dram_tensor("srcT", [H, plan['WS']], BF16, kind="ExternalInput").ap()
    aps['maskb'] = nc.dram_tensor("maskb", [128, plan['WM']], F32, kind="ExternalInput").ap()
    aps['WqT'] = nc.dram_tensor("WqT", [H, C], BF16, kind="ExternalInput").ap()
    aps['WkT'] = nc.dram_tensor("WkT", [H, C], BF16, kind="ExternalInput").ap()
    aps['WvT'] = nc.dram_tensor("WvT", [H, C], BF16, kind="ExternalInput").ap()
    aps['W1T'] = nc.dram_tensor("W1T", [H + C, H], BF16, kind="ExternalInput").ap()
    aps['W2T'] = nc.dram_tensor("W2T", [H, H], BF16, kind="ExternalInput").ap()
    aps['pbias'] = nc.dram_tensor("pbias", [128, NPB], F32, kind="ExternalInput").ap()
    aps['onespad'] = nc.dram_tensor("onespad", [128, HEADS * C], BF16, kind="ExternalInput").ap()
    aps['eye'] = nc.dram_tensor("eye", [128, 128], BF16, kind="ExternalInput").ap()
    aps['outT'] = nc.dram_tensor("outT", [H, plan['WD']], BF16, kind="ExternalOutput").ap()
    return aps


def build_body(ctx: ExitStack, tc: tile.TileContext, aps, plan):
    nc = tc.nc
    slots = plan['slots']
    ND, NS, CH = plan['ND'], plan['NS'], plan['CH']
    doff, soff, moff = plan['doff'], plan['soff'], plan['moff']
    NDmax = int(ND.max())
    NSmax = int(NS.max())
    CHmax = int(CH.max())

    wp = ctx.enter_context(tc.tile_pool(name="wp", bufs=1))
    inp = ctx.enter_context(tc.tile_pool(name="inp", bufs=3))
    act = ctx.enter_context(tc.tile_pool(name="act", bufs=1))
    # PSUM: gpp(proj q/k + v-direct) 2 + sc 2 + msg 1 + den 1 + mlp 2 = 8 banks
    gpp = ctx.enter_context(tc.tile_pool(name="gpp", bufs=2, space="PSUM"))
    scp = ctx.enter_context(tc.tile_pool(name="scp", bufs=2, space="PSUM"))
    mdp = ctx.enter_context(tc.tile_pool(name="mdp", bufs=1, space="PSUM"))
    mlp = ctx.enter_context(tc.tile_pool(name="mlp", bufs=2, space="PSUM"))

    # --- weights (persistent) ---
    wq = wp.tile([128, 2, C], BF16, tag="wq")
    wk = wp.tile([128, 2, C], BF16, tag="wk")
    wv = wp.tile([128, 2, C], BF16, tag="wv")
    w1 = wp.tile([128, 3, H], BF16, tag="w1")
    w2 = wp.tile([128, 2, H], BF16, tag="w2")
    pb = wp.tile([128, NPB], F32, tag="pb")
    maskb_t = wp.tile([128, plan['WM']], F32, tag="maskb")
    onespad = wp.tile([128, HEADS, C], BF16, tag="onespad")
    eye = wp.tile([128, 128], BF16, tag="eye")
    for a in range(2):
        nc.scalar.dma_start(out=wq[:, a, :], in_=aps['WqT'][a * 128:(a + 1) * 128, :])
        nc.scalar.dma_start(out=wk[:, a, :], in_=aps['WkT'][a * 128:(a + 1) * 128, :])
        nc.scalar.dma_start(out=wv[:, a, :], in_=aps['WvT'][a * 128:(a + 1) * 128, :])
        nc.scalar.dma_start(out=w2[:, a, :], in_=aps['W2T'][a * 128:(a + 1) * 128, :])
    for a in range(3):
        nc.scalar.dma_start(out=w1[:, a, :], in_=aps['W1T'][a * 128:(a + 1) * 128, :])
    nc.scalar.dma_start(out=pb[:], in_=aps['pbias'][:])
    nc.scalar.dma_start(out=maskb_t[:], in_=aps['maskb'][:])
    nc.scalar.dma_start(out=onespad[:], in_=aps['onespad'].rearrange("p (h c) -> p h c", h=HEADS))
    nc.scalar.dma_start(out=eye[:], in_=aps['eye'][:])

    # --- persistent V slots (zero-padded band layout), zero-filled once ---
    v_slots = []
    for i in range(3):
        vs = act.tile([128, CHmax, HEADS, C], BF16, tag=f"Vs{i}", name=f"Vs{i}")
        nc.vector.memset(vs[:], 0.0)
        v_slots.append(vs)

    # --- input tiles: 3-buffer rotation, 2-slot DMA lookahead ---
    dst_t = [None] * slots
    src_t = [None] * slots

    def load_slot(s):
        dt_ = [inp.tile([128, ND[s]], BF16, tag=f"dst{a}", name=f"dst{a}_{s}")
               for a in range(2)]
        st_ = [inp.tile([128, NS[s]], BF16, tag=f"src{a}", name=f"src{a}_{s}")
               for a in range(2)]
        for a in range(2):
            nc.sync.dma_start(out=dt_[a][:],
                              in_=aps['dstT'][a * 128:(a + 1) * 128, doff[s]:doff[s] + ND[s]])
        for a in range(2):
            nc.sync.dma_start(out=st_[a][:],
                              in_=aps['srcT'][a * 128:(a + 1) * 128, soff[s]:soff[s] + NS[s]])
        dst_t[s] = dt_
        src_t[s] = st_

    load_slot(0)
    if slots > 1:
        load_slot(1)

    for s in range(slots):
        nd_, ns_, ch = int(ND[s]), int(NS[s]), int(CH[s])
        if s + 2 < slots:
            load_slot(s + 2)
        dt_, st_ = dst_t[s], src_t[s]

        # ---------- projections ----------
        ps_q = gpp.tile([128, NDmax], F32, tag="gpp", name=f"psq{s}")
        for a in range(2):
            nc.tensor.matmul(ps_q[:, :nd_], wq[:, a, :], dt_[a][:],
                             start=(a == 0), stop=(a == 1))
        q_t = act.tile([128, NDmax], BF16, tag="q", name=f"q{s}", bufs=2)
        nc.vector.tensor_scalar_add(q_t[:, :nd_], ps_q[:, :nd_], pb[:, 0:1])

        ps_k = gpp.tile([128, NSmax], F32, tag="gpp", name=f"psk{s}")
        for a in range(2):
            nc.tensor.matmul(ps_k[:, :ns_], wk[:, a, :], st_[a][:],
                             start=(a == 0), stop=(a == 1))
        k_t = act.tile([128, NSmax], BF16, tag="k", name=f"k{s}", bufs=2)
        nc.vector.tensor_scalar_add(k_t[:, :ns_], ps_k[:, :ns_], pb[:, 1:2])

        # ---------- V direct (natural [tok, chan]); bias folded into msgn ----------
        ps_vd = gpp.tile([128, CHmax, 128], F32, tag="gpp", name=f"psvd{s}")
        for j in range(ch):
            for a in range(2):
                nc.tensor.matmul(ps_vd[:, j, :],
                                 st_[a][:, j * 128:(j + 1) * 128], wv[:, a, :],
                                 start=(a == 0), stop=(a == 1))
        v_sb = v_slots[s % 3]
        vdst = bass.AP(tensor=v_sb.tensor, offset=v_sb.offset,
                       ap=[v_sb.ap[0]] + [[HEADS * C, ch], [C + DH, HEADS], [1, DH]])
        vsrc = bass.AP(tensor=ps_vd.tensor, offset=ps_vd.offset,
                       ap=[ps_vd.ap[0]] + [[128, ch], [DH, HEADS], [1, DH]])
        nc.vector.tensor_copy(vdst, vsrc)

        # ---------- attention ----------
        ps_msg = mdp.tile([128, NDmax], F32, tag="msg", name=f"psmsg{s}")
        ps_den = mdp.tile([128, NDmax], F32, tag="den", name=f"psden{s}")
        for j in range(ch):
            for h in range(HEADS):
                ps_sc = scp.tile([128, 512], F32, tag="sc", name=f"pssc{s}_{j}_{h}")
                nc.tensor.matmul(
                    ps_sc[:, :nd_],
                    k_t[32 * h:32 * h + 32, j * 128:(j + 1) * 128],
                    q_t[32 * h:32 * h + 32, :nd_],
                    start=True, stop=True, tile_position=(32 * h, 0))
                e_t = act.tile([128, NDmax], BF16, tag="E", name=f"E{s}_{j}_{h}", bufs=6)
                nc.scalar.activation(e_t[:, :nd_], ps_sc[:, :nd_],
                                     mybir.ActivationFunctionType.Exp,
                                     bias=maskb_t[:, moff[s] + j: moff[s] + j + 1])
                first = (j == 0 and h == 0)
                last = (j == ch - 1 and h == 3)
                nc.tensor.matmul(
                    ps_msg[:, :nd_],
                    v_sb[:, j, h, :],
                    e_t[:, :nd_],
                    start=first, stop=last)
                nc.tensor.matmul(
                    ps_den[:, :nd_],
                    onespad[:, h, :],
                    e_t[:, :nd_],
                    start=first, stop=last)
        r_sb = act.tile([128, NDmax], F32, tag="rsb", name=f"rsb{s}", bufs=2)
        nc.vector.reciprocal_approx_fast(r_sb[:, :nd_], ps_den[:, :nd_])
        msgn = act.tile([128, NDmax], BF16, tag="msgn", name=f"msgn{s}", bufs=2)
        nc.vector.tensor_mul(msgn[:, :nd_], ps_msg[:, :nd_], r_sb[:, :nd_])

        # ---------- MLP (merge folded into W1; residual via eye matmul) ----------
        y1 = [None, None]
        for o in range(2):
            ps_y = mlp.tile([128, NDmax], F32, tag="mlp", name=f"psy{s}_{o}")
            rhs_list = [dt_[0][:], dt_[1][:], msgn[:, :nd_]]
            for kk in range(3):
                nc.tensor.matmul(ps_y[:, :nd_], w1[:, kk, o * 128:(o + 1) * 128],
                                 rhs_list[kk], start=(kk == 0), stop=(kk == 2))
            y1_t = act.tile([128, NDmax], BF16, tag=f"y1_{o}", name=f"y1_{s}_{o}", bufs=2)
            nc.vector.tensor_scalar(y1_t[:, :nd_], ps_y[:, :nd_], pb[:, 2 + o:3 + o], 0.0,
                                    op0=mybir.AluOpType.add, op1=mybir.AluOpType.max)
            y1[o] = y1_t
        for o in range(2):
            ps_z = mlp.tile([128, NDmax], F32, tag="mlp", name=f"psz{s}_{o}")
            for kk in range(2):
                nc.tensor.matmul(ps_z[:, :nd_], w2[:, kk, o * 128:(o + 1) * 128],
                                 y1[kk][:, :nd_], start=(kk == 0), stop=False)
            nc.tensor.matmul(ps_z[:, :nd_], eye[:], dt_[o][:],
                             start=False, stop=True)
            out_sb = act.tile([128, NDmax], BF16, tag=f"out_{o}", name=f"out_{s}_{o}", bufs=2)
            nc.vector.tensor_scalar_add(out_sb[:, :nd_], ps_z[:, :nd_], pb[:, 4 + o:5 + o])
            nc.sync.dma_start(out=aps['outT'][o * 128:(o + 1) * 128, doff[s]:doff[s] + nd_],
                                in_=out_sb[:, :nd_])


def build_nc(plan=None):
    if plan is None:
        plan = _PLAN
    nc = bacc.Bacc("TRN2", target_bir_lowering=False, debug=False,
                   enable_asserts=True, num_devices=NCORES)
    aps = declare_tensors(nc, plan)
    with tile.TileContext(nc) as tc:
        with ExitStack() as ctx:
            build_body(ctx, tc, aps, plan)
    nc.compile()
    return nc


def in_map(core, shared):
    m = dict(dstT=core['dstT'], srcT=core['srcT'], maskb=core['maskb'])
    m.update({k: shared[k] for k in ('WqT', 'WkT', 'WvT', 'W1T', 'W2T',
                                     'pbias', 'onespad', 'eye')})
    return m


def assemble(outTs, meta):
    nd = meta['nd']
    doff_g = meta['doff_g']
    plan = meta['plan']
    out = np.empty((int(nd.sum()), H), np.float32)
    for c in range(NCORES):
        for j in range(plan['slots']):
            g = plan['assign'][c, j]
            sl = outTs[c][:, plan['doff'][j]: plan['doff'][j] + nd[g]]
            out[doff_g[g]:doff_g[g] + nd[g]] = sl.T.astype(np.float32)
    return out


def kernel(**inputs):
    cores, shared, meta = host_prep(inputs)
    nc = build_nc(meta['plan'])
    in_maps = [in_map(cores[c], shared) for c in range(NCORES)]
    res = run_bass_kernel_spmd(nc, in_maps, core_ids=list(range(NCORES)))
    outTs = [np.asarray(res.results[c]["outT"]) for c in range(NCORES)]
    return assemble(outTs, meta)


# revision 12
# speedup vs baseline: 2.2511x; 1.0484x over previous
"""Trainium2 Bass kernel for nn_CrossAttentionLayer (ragged cross-attention + MLP).

Sharding: 64 ragged segments -> 8 cores x 8 slots. Segments are sorted by
(src-chunk count, dst count) and dealt so the 8 segments sharing a slot have
similar sizes; each slot is trimmed to the max dst count (ND) / src count
(NS, rounded to 128) over its 8 segments, so all cores run one SPMD program.

All matmul operands are bf16 (fp32 PSUM accumulation). Activations stay
channel-major [chan, tok]; softmax runs in scoresT orientation [src, dst]
with the src-padding mask applied as a per-partition bias on the exp. The
denominator is computed with banded-ones matmuls replicating each head's
denominator across its 32 partitions; normalization is reciprocal_approx_fast
+ multiply. V is produced directly in natural [tok, chan] orientation (src
chunks as stationary), with its bias folded into the normalization:
msgn = msg * recip(den) + bv. The dst residual is added via an identity
matmul accumulated into the MLP2 PSUM group. Merge conv + BN are folded into
the MLP weights on the host.
"""
import math
import sys
from contextlib import ExitStack

import numpy as np
import ml_dtypes

try:
    import concourse.bass as bass
except ImportError:
    sys.path.insert(0, "/opt/trn_rl_repo")
    import concourse.bass as bass

import concourse.tile as tile
from concourse import bacc, mybir
from concourse.bass_utils import run_bass_kernel_spmd

F32 = mybir.dt.float32
F32R = mybir.dt.float32r
BF16 = mybir.dt.bfloat16
BF = ml_dtypes.bfloat16

H = 256          # h_dim
C = 128          # h_div
HEADS = 4
DH = 32
NCORES = 8
NPB = 7          # per-partition bias columns: bq,bk,b1a,b1b,b2a,b2b,bv
MASK_NEG = -20000.0

# Filled by host_prep; read by build_nc/in_map (same process).
_PLAN = {}


def _make_plan(nd, ns):
    """Assign 64 segments to (core, slot); compute per-slot widths."""
    B = len(nd)
    slots = B // NCORES
    chunks_of = np.ceil(ns / 128).astype(int)
    # sort by (chunks desc, nd desc); deal consecutive groups of 8 per slot,
    # then hill-climb pair swaps on the PE-cycle objective
    order = list(np.lexsort((-nd, -chunks_of)))

    def slot_cost(grp):
        ndm = -(-max(nd[g] for g in grp) // 4) * 4
        chm = max(chunks_of[g] for g in grp)
        return ndm * (14 + 12 * chm) + 2 * 128 * chm + 256 * chm

    rng = np.random.default_rng(0)
    groups = [order[j * NCORES:(j + 1) * NCORES] for j in range(slots)]
    costs = [slot_cost(g) for g in groups]
    for _ in range(30000):
        j1, j2 = rng.integers(0, slots, 2)
        if j1 == j2:
            continue
        i1, i2 = rng.integers(0, NCORES, 2)
        g1, g2 = groups[j1][i1], groups[j2][i2]
        groups[j1][i1], groups[j2][i2] = g2, g1
        c1, c2 = slot_cost(groups[j1]), slot_cost(groups[j2])
        if c1 + c2 < costs[j1] + costs[j2]:
            costs[j1], costs[j2] = c1, c2
        else:
            groups[j1][i1], groups[j2][i2] = g1, g2
    # big slots first so the tail slot is the cheapest
    sidx = sorted(range(slots), key=lambda j: -costs[j])
    groups = [groups[j] for j in sidx]

    assign = np.empty((NCORES, slots), dtype=int)   # (core, slot) -> segment
    ND = np.empty(slots, dtype=int)
    NS = np.empty(slots, dtype=int)
    for j in range(slots):
        grp = groups[j]
        for c in range(NCORES):
            assign[c, j] = grp[c]
        ND[j] = int(-(-max(nd[g] for g in grp) // 4) * 4)       # mult of 4
        NS[j] = int(-(-max(ns[g] for g in grp) // 128) * 128)   # mult of 128
    CH = (NS // 128).astype(int)
    return dict(slots=slots, assign=assign, ND=ND, NS=NS, CH=CH,
                doff=np.concatenate([[0], np.cumsum(ND)[:-1]]),
                soff=np.concatenate([[0], np.cumsum(NS)[:-1]]),
                moff=np.concatenate([[0], np.cumsum(CH)[:-1]]),
                WD=int(ND.sum()), WS=int(NS.sum()), WM=int(CH.sum()))


def host_prep(inputs):
    src_h = np.asarray(inputs['src_h'], np.float32)
    dst_h = np.asarray(inputs['dst_h'], np.float32)
    ns = np.asarray(inputs['src_num_verts']).astype(np.int64)
    nd = np.asarray(inputs['dst_num_verts']).astype(np.int64)
    soff_g = np.concatenate([[0], np.cumsum(ns)[:-1]])
    doff_g = np.concatenate([[0], np.cumsum(nd)[:-1]])

    plan = _make_plan(nd, ns)
    global _PLAN
    _PLAN = plan
    slots = plan['slots']

    perm = np.empty(C, np.int64)
    for chat in range(C):
        h, d = divmod(chat, DH)
        perm[chat] = d * HEADS + h
    s = 1.0 / math.sqrt(DH)

    f32 = lambda k: np.asarray(inputs[k], np.float32)
    Wq, bq = f32('Wq'), f32('bq')
    Wk, bk = f32('Wk'), f32('bk')
    Wv, bv = f32('Wv'), f32('bv')
    Wm, bm = f32('Wm'), f32('bm')
    W1, b1 = f32('W1'), f32('b1')
    W2, b2 = f32('W2'), f32('b2')
    g1, be1, rm1, rv1 = f32('g1'), f32('be1'), f32('rm1'), f32('rv1')
    g2, be2, rm2, rv2 = f32('g2'), f32('be2'), f32('rm2'), f32('rv2')

    WqT = np.ascontiguousarray((Wq[perm] * s).T)          # [256,128]
    bq_s = bq[perm] * s
    WkT = np.ascontiguousarray(Wk[perm].T)
    bk_r = bk[perm]
    WvT = np.ascontiguousarray(Wv[perm].T)                # [256,128] moving for V-direct
    bv_r = bv[perm]
    Wm_p = Wm[:, perm]
    a1 = g1 / np.sqrt(rv1 + 1e-5)
    W1_f = W1 * a1[:, None]
    b1_f = b1 * a1 + be1 - rm1 * a1
    a2 = g2 / np.sqrt(rv2 + 1e-5)
    W2_f = W2 * a2[:, None]
    b2_f = b2 * a2 + be2 - rm2 * a2
    W1m_p = W1_f[:, H:] @ Wm_p
    # V bias folded all the way into the MLP1 bias: msg enters MLP1 as
    # msg/den (no bias), and W1m_p @ bv_perm is a constant.
    b1_p = b1_f + W1_f[:, H:] @ bm + W1m_p @ bv_r
    W1T = np.ascontiguousarray(np.concatenate([W1_f[:, :H], W1m_p], axis=1).T)  # [384,256]
    W2T = np.ascontiguousarray(W2_f.T)                    # [256,256]

    pbias = np.zeros((128, NPB), np.float32)
    pbias[:, 0] = bq_s
    pbias[:, 1] = bk_r
    pbias[:, 2] = b1_p[:128]
    pbias[:, 3] = b1_p[128:]
    pbias[:, 4] = b2_f[:128]
    pbias[:, 5] = b2_f[128:]

    ND, NS, CH = plan['ND'], plan['NS'], plan['CH']
    doff, soff, moff = plan['doff'], plan['soff'], plan['moff']
    cores = []
    for c in range(NCORES):
        dstT = np.zeros((H, plan['WD']), BF)
        srcT = np.zeros((H, plan['WS']), BF)
        maskb = np.full((128, plan['WM']), MASK_NEG, np.float32)
        for j in range(slots):
            g = plan['assign'][c, j]
            dstT[:, doff[j]:doff[j] + nd[g]] = dst_h[doff_g[g]:doff_g[g] + nd[g]].T.astype(BF)
            srcT[:, soff[j]:soff[j] + ns[g]] = src_h[soff_g[g]:soff_g[g] + ns[g]].T.astype(BF)
            for jj in range(CH[j]):
                valid = max(0, min(128, int(ns[g]) - jj * 128))
                maskb[:valid, moff[j] + jj] = 0.0
        cores.append(dict(dstT=dstT, srcT=srcT, maskb=maskb))

    onespad = np.zeros((128, HEADS, C), BF)
    for h in range(HEADS):
        onespad[:, h, h * DH:(h + 1) * DH] = 1.0
    shared = dict(
        WqT=WqT.astype(BF), WkT=WkT.astype(BF), WvT=WvT.astype(BF),
        W1T=W1T.astype(BF), W2T=W2T.astype(BF), pbias=pbias,
        onespad=onespad.reshape(128, HEADS * C),
        eye=np.eye(128, dtype=BF),
    )
    meta = dict(nd=nd, doff_g=doff_g, plan=plan)
    return cores, shared, meta


def declare_tensors(nc, plan):
    aps = {}
    aps['dstT'] = nc.dram_tensor("dstT", [H, plan['WD']], BF16, kind="ExternalInput").ap()
    aps['srcT'] = nc.dram_tensor("srcT", [H, plan['WS']], BF16, kind="ExternalInput").ap()
    aps['maskb'] = nc.dram_tensor("maskb", [128, plan['WM']], F32, kind="ExternalInput").ap()
    aps['WqT'] = nc.dram_tensor("WqT", [H, C], BF16, kind="ExternalInput").ap()
    aps['WkT'] = nc.dram_tensor("WkT", [H, C], BF16, kind="ExternalInput").ap()
    aps['WvT'] = nc.dram_tensor("WvT", [H, C], BF16, kind="ExternalInput").ap()
    aps['W1T'] = nc.dram_tensor("W1T", [H + C, H], BF16, kind="ExternalInput").ap()
    aps['W2T'] = nc.dram_tensor("W2T", [H, H], BF16, kind="ExternalInput").ap()
    aps['pbias'] = nc.dram_tensor("pbias", [128, NPB], F32, kind="ExternalInput").ap()
    aps['onespad'] = nc.dram_tensor("onespad", [128, HEADS * C], BF16, kind="ExternalInput").ap()
    aps['eye'] = nc.dram_tensor("eye", [128, 128], BF16, kind="ExternalInput").ap()
    aps['outT'] = nc.dram_tensor("outT", [H, plan['WD']], BF16, kind="ExternalOutput").ap()
    return aps


def build_body(ctx: ExitStack, tc: tile.TileContext, aps, plan):
    nc = tc.nc
    slots = plan['slots']
    ND, NS, CH = plan['ND'], plan['NS'], plan['CH']
    doff, soff, moff = plan['doff'], plan['soff'], plan['moff']
    NDmax = int(ND.max())
    NSmax = int(NS.max())
    CHmax = int(CH.max())

    wp = ctx.enter_context(tc.tile_pool(name="wp", bufs=1))
    inp = ctx.enter_context(tc.tile_pool(name="inp", bufs=3))
    act = ctx.enter_context(tc.tile_pool(name="act", bufs=1))
    # PSUM: gpp(proj q/k + v-direct) 2 + sc 2 + msg 1 + den 1 + mlp 2 = 8 banks
    gpp = ctx.enter_context(tc.tile_pool(name="gpp", bufs=2, space="PSUM"))
    scp = ctx.enter_context(tc.tile_pool(name="scp", bufs=2, space="PSUM"))
    mdp = ctx.enter_context(tc.tile_pool(name="mdp", bufs=1, space="PSUM"))
    mlp = ctx.enter_context(tc.tile_pool(name="mlp", bufs=2, space="PSUM"))

    # --- weights (persistent) ---
    wq = wp.tile([128, 2, C], BF16, tag="wq")
    wk = wp.tile([128, 2, C], BF16, tag="wk")
    wv = wp.tile([128, 2, C], BF16, tag="wv")
    w1 = wp.tile([128, 3, H], BF16, tag="w1")
    w2 = wp.tile([128, 2, H], BF16, tag="w2")
    pb = wp.tile([128, NPB], F32, tag="pb")
    maskb_t = wp.tile([128, plan['WM']], F32, tag="maskb")
    onespad = wp.tile([128, HEADS, C], BF16, tag="onespad")
    eye = wp.tile([128, 128], BF16, tag="eye")
    for a in range(2):
        nc.scalar.dma_start(out=wq[:, a, :], in_=aps['WqT'][a * 128:(a + 1) * 128, :])
        nc.scalar.dma_start(out=wk[:, a, :], in_=aps['WkT'][a * 128:(a + 1) * 128, :])
        nc.scalar.dma_start(out=wv[:, a, :], in_=aps['WvT'][a * 128:(a + 1) * 128, :])
    nc.scalar.dma_start(out=pb[:], in_=aps['pbias'][:])
    nc.scalar.dma_start(out=maskb_t[:], in_=aps['maskb'][:])
    nc.scalar.dma_start(out=onespad[:], in_=aps['onespad'].rearrange("p (h c) -> p h c", h=HEADS))
    for a in range(3):
        nc.scalar.dma_start(out=w1[:, a, :], in_=aps['W1T'][a * 128:(a + 1) * 128, :])
    for a in range(2):
        nc.scalar.dma_start(out=w2[:, a, :], in_=aps['W2T'][a * 128:(a + 1) * 128, :])
    nc.scalar.dma_start(out=eye[:], in_=aps['eye'][:])

    # --- persistent V slots (zero-padded band layout), zero-filled once ---
    v_slots = []
    for i in range(3):
        vs = act.tile([128, CHmax, HEADS, C], BF16, tag=f"Vs{i}", name=f"Vs{i}")
        nc.vector.memset(vs[:], 0.0)
        v_slots.append(vs)

    # --- input tiles: 3-buffer rotation, 2-slot DMA lookahead ---
    dst_t = [None] * slots
    src_t = [None] * slots

    def load_slot(s):
        dt_ = [inp.tile([128, ND[s]], BF16, tag=f"dst{a}", name=f"dst{a}_{s}")
               for a in range(2)]
        st_ = [inp.tile([128, NS[s]], BF16, tag=f"src{a}", name=f"src{a}_{s}")
               for a in range(2)]
        for a in range(2):
            nc.sync.dma_start(out=dt_[a][:],
                              in_=aps['dstT'][a * 128:(a + 1) * 128, doff[s]:doff[s] + ND[s]])
        for a in range(2):
            nc.sync.dma_start(out=st_[a][:],
                              in_=aps['srcT'][a * 128:(a + 1) * 128, soff[s]:soff[s] + NS[s]])
        dst_t[s] = dt_
        src_t[s] = st_

    load_slot(0)
    if slots > 1:
        load_slot(1)

    for s in range(slots):
        nd_, ns_, ch = int(ND[s]), int(NS[s]), int(CH[s])
        if s + 2 < slots:
            load_slot(s + 2)
        dt_, st_ = dst_t[s], src_t[s]

        # ---------- projections ----------
        ps_q = gpp.tile([128, NDmax], F32, tag="gpp", name=f"psq{s}")
        for a in range(2):
            nc.tensor.matmul(ps_q[:, :nd_], wq[:, a, :], dt_[a][:],
                             start=(a == 0), stop=(a == 1))
        q_t = act.tile([128, NDmax], BF16, tag="q", name=f"q{s}", bufs=2)
        nc.vector.tensor_scalar_add(q_t[:, :nd_], ps_q[:, :nd_], pb[:, 0:1])

        ps_k = gpp.tile([128, NSmax], F32, tag="gpp", name=f"psk{s}")
        for a in range(2):
            nc.tensor.matmul(ps_k[:, :ns_], wk[:, a, :], st_[a][:],
                             start=(a == 0), stop=(a == 1))
        k_t = act.tile([128, NSmax], BF16, tag="k", name=f"k{s}", bufs=2)
        nc.vector.tensor_scalar_add(k_t[:, :ns_], ps_k[:, :ns_], pb[:, 1:2])

        # ---------- V direct (natural [tok, chan]); bias folded into msgn ----------
        ps_vd = gpp.tile([128, CHmax, 128], F32, tag="gpp", name=f"psvd{s}")
        for j in range(ch):
            for a in range(2):
                nc.tensor.matmul(ps_vd[:, j, :],
                                 st_[a][:, j * 128:(j + 1) * 128], wv[:, a, :],
                                 start=(a == 0), stop=(a == 1))
        v_sb = v_slots[s % 3]
        vdst = bass.AP(tensor=v_sb.tensor, offset=v_sb.offset,
                       ap=[v_sb.ap[0]] + [[HEADS * C, ch], [C + DH, HEADS], [1, DH]])
        vsrc = bass.AP(tensor=ps_vd.tensor, offset=ps_vd.offset,
                       ap=[ps_vd.ap[0]] + [[128, ch], [DH, HEADS], [1, DH]])
        nc.vector.tensor_copy(vdst, vsrc)

        # ---------- attention ----------
        ps_msg = mdp.tile([128, NDmax], F32, tag="msg", name=f"psmsg{s}")
        ps_den = mdp.tile([128, NDmax], F32, tag="den", name=f"psden{s}")
        e0 = [None] * HEADS     # head -> first chunk's e tile
        e_run = [None] * HEADS   # head -> chunk-sum accumulator (DVE)
        for j in range(ch):
            for h in range(HEADS):
                ps_sc = scp.tile([128, 512], F32, tag="sc", name=f"pssc{s}_{j}_{h}")
                nc.tensor.matmul(
                    ps_sc[:, :nd_],
                    k_t[32 * h:32 * h + 32, j * 128:(j + 1) * 128],
                    q_t[32 * h:32 * h + 32, :nd_],
                    start=True, stop=True, tile_position=(32 * h, 0))
                e_t = act.tile([128, NDmax], BF16, tag="E", name=f"E{s}_{j}_{h}", bufs=6)
                nc.scalar.activation(e_t[:, :nd_], ps_sc[:, :nd_],
                                     mybir.ActivationFunctionType.Exp,
                                     bias=maskb_t[:, moff[s] + j: moff[s] + j + 1])
                nc.tensor.matmul(
                    ps_msg[:, :nd_],
                    v_sb[:, j, h, :],
                    e_t[:, :nd_],
                    start=(j == 0 and h == 0), stop=(j == ch - 1 and h == 3))
                # denominator: sum e over src chunks on DVE, one matmul per
                # head at the end (saves (ch-1)*4 PE matmuls per slot)
                if ch == 1:
                    e_run[h] = e_t
                elif j == 0:
                    e0[h] = e_t
                elif j == 1:
                    er = act.tile([128, NDmax], BF16, tag=f"er{h}", name=f"er{s}_{h}", bufs=2)
                    nc.vector.tensor_add(er[:, :nd_], e0[h][:, :nd_], e_t[:, :nd_])
                    e_run[h] = er
                else:
                    nc.vector.tensor_add(e_run[h][:, :nd_], e_run[h][:, :nd_],
                                         e_t[:, :nd_])
        for h in range(HEADS):
            nc.tensor.matmul(
                ps_den[:, :nd_],
                onespad[:, h, :],
                e_run[h][:, :nd_],
                start=(h == 0), stop=(h == 3))
        r_sb = act.tile([128, NDmax], F32, tag="rsb", name=f"rsb{s}", bufs=2)
        nc.vector.reciprocal_approx_fast(r_sb[:, :nd_], ps_den[:, :nd_])
        msgn = act.tile([128, NDmax], BF16, tag="msgn", name=f"msgn{s}", bufs=2)
        nc.vector.tensor_mul(msgn[:, :nd_], ps_msg[:, :nd_], r_sb[:, :nd_])

        # ---------- MLP (merge folded into W1; residual via eye matmul) ----------
        y1 = [None, None]
        for o in range(2):
            ps_y = mlp.tile([128, NDmax], F32, tag="mlp", name=f"psy{s}_{o}")
            rhs_list = [dt_[0][:], dt_[1][:], msgn[:, :nd_]]
            for kk in range(3):
                nc.tensor.matmul(ps_y[:, :nd_], w1[:, kk, o * 128:(o + 1) * 128],
                                 rhs_list[kk], start=(kk == 0), stop=(kk == 2))
            y1_t = act.tile([128, NDmax], BF16, tag=f"y1_{o}", name=f"y1_{s}_{o}", bufs=2)
            nc.vector.tensor_scalar(y1_t[:, :nd_], ps_y[:, :nd_], pb[:, 2 + o:3 + o], 0.0,
                                    op0=mybir.AluOpType.add, op1=mybir.AluOpType.max)
            y1[o] = y1_t
        for o in range(2):
            ps_z = mlp.tile([128, NDmax], F32, tag="mlp", name=f"psz{s}_{o}")
            for kk in range(2):
                nc.tensor.matmul(ps_z[:, :nd_], w2[:, kk, o * 128:(o + 1) * 128],
                                 y1[kk][:, :nd_], start=(kk == 0), stop=False)
            nc.tensor.matmul(ps_z[:, :nd_], eye[:], dt_[o][:],
                             start=False, stop=True)
            out_sb = act.tile([128, NDmax], BF16, tag=f"out_{o}", name=f"out_{s}_{o}", bufs=2)
            nc.vector.tensor_scalar_add(out_sb[:, :nd_], ps_z[:, :nd_], pb[:, 4 + o:5 + o])
            nc.sync.dma_start(out=aps['outT'][o * 128:(o + 1) * 128, doff[s]:doff[s] + nd_],
                                in_=out_sb[:, :nd_])


def build_nc(plan=None):
    if plan is None:
        plan = _PLAN
    nc = bacc.Bacc("TRN2", target_bir_lowering=False, debug=False,
                   enable_asserts=True, num_devices=NCORES)
    aps = declare_tensors(nc, plan)
    with tile.TileContext(nc) as tc:
        with ExitStack() as ctx:
            build_body(ctx, tc, aps, plan)
    nc.compile()
    return nc


def in_map(core, shared):
    m = dict(dstT=core['dstT'], srcT=core['srcT'], maskb=core['maskb'])
    m.update({k: shared[k] for k in ('WqT', 'WkT', 'WvT', 'W1T', 'W2T',
                                     'pbias', 'onespad', 'eye')})
    return m


def assemble(outTs, meta):
    nd = meta['nd']
    doff_g = meta['doff_g']
    plan = meta['plan']
    out = np.empty((int(nd.sum()), H), np.float32)
    for c in range(NCORES):
        for j in range(plan['slots']):
            g = plan['assign'][c, j]
            sl = outTs[c][:, plan['doff'][j]: plan['doff'][j] + nd[g]]
            out[doff_g[g]:doff_g[g] + nd[g]] = sl.T.astype(np.float32)
    return out


def kernel(**inputs):
    cores, shared, meta = host_prep(inputs)
    nc = build_nc(meta['plan'])
    in_maps = [in_map(cores[c], shared) for c in range(NCORES)]
    res = run_bass_kernel_spmd(nc, in_maps, core_ids=list(range(NCORES)))
    outTs = [np.asarray(res.results[c]["outT"]) for c in range(NCORES)]
    return assemble(outTs, meta)


# revision 13
# speedup vs baseline: 2.4980x; 1.1097x over previous
"""Trainium2 Bass kernel for nn_CrossAttentionLayer (ragged cross-attention + MLP).

Sharding: 64 ragged segments -> 8 cores x 8 slots. Segments are sorted by
(src-chunk count, dst count), dealt into slots of 8, then hill-climbed so
segments sharing a slot have similar sizes; each slot is trimmed to the max
dst count (ND) / src count (NS, rounded to 128) over its 8 segments, so all
cores run one SPMD program.

All matmul operands are bf16 (fp32 PSUM accumulation). Activations stay
channel-major [chan, tok]; softmax runs in scoresT orientation [src, dst]
with the src-padding mask applied as a per-partition bias on the exp. The
denominator is computed by summing e over src chunks on DVE/GpSimd, then one
banded-ones matmul per head replicates each head's denominator across its 32
partitions; normalization is reciprocal_approx_fast + multiply. V is
produced directly in natural [tok, chan] orientation (src chunks as
stationary); its bias and the merge conv + BN are folded into the MLP
weights on the host. The dst residual is added via an identity matmul
accumulated into the MLP2 PSUM group.

All weights/biases/masks ship in one packed bf16 DMA (f32 parts bitcast);
per-slot inputs/outputs move as single 3D-AP DMAs with a 3-buffer rotation
and 2-slot prefetch.
"""
import math
import sys
from contextlib import ExitStack

import numpy as np
import ml_dtypes

try:
    import concourse.bass as bass
except ImportError:
    sys.path.insert(0, "/opt/trn_rl_repo")
    import concourse.bass as bass

import concourse.tile as tile
from concourse import bacc, mybir
from concourse.bass_utils import run_bass_kernel_spmd

F32 = mybir.dt.float32
BF16 = mybir.dt.bfloat16
BF = ml_dtypes.bfloat16

H = 256          # h_dim
C = 128          # h_div
HEADS = 4
DH = 32
NCORES = 8
MASK_NEG = -20000.0

# packed-weights column offsets (bf16 elements per partition)
OFF_WQ = 0
OFF_WK = 256
OFF_WV = 512
OFF_W1 = 768
OFF_W2 = 1536
OFF_ONES = 2048
OFF_EYE = 2560
OFF_PB = 2688    # 6 f32 = 12 bf16
OFF_MASK = 2700  # WM f32 = 2*WM bf16

# Filled by host_prep; read by build_nc/in_map (same process).
_PLAN = {}


def _make_plan(nd, ns):
    """Assign 64 segments to (core, slot); compute per-slot widths."""
    B = len(nd)
    slots = B // NCORES
    chunks_of = np.ceil(ns / 128).astype(int)
    order = list(np.lexsort((-nd, -chunks_of)))

    def slot_cost(grp):
        ndm = -(-max(nd[g] for g in grp) // 4) * 4
        chm = max(chunks_of[g] for g in grp)
        return ndm * (14 + 8 * chm + 4) + 2 * 128 * chm + 256 * chm

    rng = np.random.default_rng(0)
    groups = [order[j * NCORES:(j + 1) * NCORES] for j in range(slots)]
    costs = [slot_cost(g) for g in groups]
    for _ in range(30000):
        j1, j2 = rng.integers(0, slots, 2)
        if j1 == j2:
            continue
        i1, i2 = rng.integers(0, NCORES, 2)
        g1, g2 = groups[j1][i1], groups[j2][i2]
        groups[j1][i1], groups[j2][i2] = g2, g1
        c1, c2 = slot_cost(groups[j1]), slot_cost(groups[j2])
        if c1 + c2 < costs[j1] + costs[j2]:
            costs[j1], costs[j2] = c1, c2
        else:
            groups[j1][i1], groups[j2][i2] = g1, g2
    # big slots first so the tail slot is the cheapest
    sidx = sorted(range(slots), key=lambda j: -costs[j])
    groups = [groups[j] for j in sidx]

    assign = np.empty((NCORES, slots), dtype=int)   # (core, slot) -> segment
    ND = np.empty(slots, dtype=int)
    NS = np.empty(slots, dtype=int)
    for j in range(slots):
        grp = groups[j]
        for c in range(NCORES):
            assign[c, j] = grp[c]
        ND[j] = int(-(-max(nd[g] for g in grp) // 4) * 4)       # mult of 4
        NS[j] = int(-(-max(ns[g] for g in grp) // 128) * 128)   # mult of 128
    CH = (NS // 128).astype(int)
    return dict(slots=slots, assign=assign, ND=ND, NS=NS, CH=CH,
                doff=np.concatenate([[0], np.cumsum(ND)[:-1]]),
                soff=np.concatenate([[0], np.cumsum(NS)[:-1]]),
                moff=np.concatenate([[0], np.cumsum(CH)[:-1]]),
                WD=int(ND.sum()), WS=int(NS.sum()), WM=int(CH.sum()))


def host_prep(inputs):
    src_h = np.asarray(inputs['src_h'], np.float32)
    dst_h = np.asarray(inputs['dst_h'], np.float32)
    ns = np.asarray(inputs['src_num_verts']).astype(np.int64)
    nd = np.asarray(inputs['dst_num_verts']).astype(np.int64)
    soff_g = np.concatenate([[0], np.cumsum(ns)[:-1]])
    doff_g = np.concatenate([[0], np.cumsum(nd)[:-1]])

    plan = _make_plan(nd, ns)
    global _PLAN
    _PLAN = plan
    slots = plan['slots']

    perm = np.empty(C, np.int64)
    for chat in range(C):
        h, d = divmod(chat, DH)
        perm[chat] = d * HEADS + h
    s = 1.0 / math.sqrt(DH)

    f32 = lambda k: np.asarray(inputs[k], np.float32)
    Wq, bq = f32('Wq'), f32('bq')
    Wk, bk = f32('Wk'), f32('bk')
    Wv, bv = f32('Wv'), f32('bv')
    Wm, bm = f32('Wm'), f32('bm')
    W1, b1 = f32('W1'), f32('b1')
    W2, b2 = f32('W2'), f32('b2')
    g1, be1, rm1, rv1 = f32('g1'), f32('be1'), f32('rm1'), f32('rv1')
    g2, be2, rm2, rv2 = f32('g2'), f32('be2'), f32('rm2'), f32('rv2')

    WqT = (Wq[perm] * s).T                                # [256,128]
    bq_s = bq[perm] * s
    WkT = Wk[perm].T
    WvT = Wv[perm].T
    bv_r = bv[perm]
    Wm_p = Wm[:, perm]
    a1 = g1 / np.sqrt(rv1 + 1e-5)
    W1_f = W1 * a1[:, None]
    b1_f = b1 * a1 + be1 - rm1 * a1
    a2 = g2 / np.sqrt(rv2 + 1e-5)
    W2_f = W2 * a2[:, None]
    b2_f = b2 * a2 + be2 - rm2 * a2
    W1m_p = W1_f[:, H:] @ Wm_p
    # V bias folded all the way into the MLP1 bias: msg enters MLP1 as
    # msg/den (no bias), and W1m_p @ bv_perm is a constant.
    b1_p = b1_f + W1_f[:, H:] @ bm + W1m_p @ bv_r
    W1T = np.concatenate([W1_f[:, :H], W1m_p], axis=1).T  # [384,256]
    W2T = W2_f.T                                          # [256,256]

    pbias = np.zeros((128, 6), np.float32)
    pbias[:, 0] = bq_s
    pbias[:, 1] = bk[perm]
    pbias[:, 2] = b1_p[:128]
    pbias[:, 3] = b1_p[128:]
    pbias[:, 4] = b2_f[:128]
    pbias[:, 5] = b2_f[128:]

    onespad = np.zeros((128, HEADS, C), BF)
    for h in range(HEADS):
        onespad[:, h, h * DH:(h + 1) * DH] = 1.0

    WM = plan['WM']
    WTOT = OFF_MASK + 2 * WM

    def pack_weights(maskb):
        wpk = np.zeros((128, WTOT), BF)
        wpk[:, OFF_WQ:OFF_WQ + 256] = WqT.reshape(2, 128, 128).transpose(1, 0, 2).reshape(128, 256).astype(BF)
        wpk[:, OFF_WK:OFF_WK + 256] = WkT.reshape(2, 128, 128).transpose(1, 0, 2).reshape(128, 256).astype(BF)
        wpk[:, OFF_WV:OFF_WV + 256] = WvT.reshape(2, 128, 128).transpose(1, 0, 2).reshape(128, 256).astype(BF)
        wpk[:, OFF_W1:OFF_W1 + 768] = W1T.reshape(3, 128, 256).transpose(1, 0, 2).reshape(128, 768).astype(BF)
        wpk[:, OFF_W2:OFF_W2 + 512] = W2T.reshape(2, 128, 256).transpose(1, 0, 2).reshape(128, 512).astype(BF)
        wpk[:, OFF_ONES:OFF_ONES + 512] = onespad.reshape(128, 512)
        wpk[:, OFF_EYE:OFF_EYE + 128] = np.eye(128, dtype=BF)
        wpk[:, OFF_PB:OFF_PB + 12] = pbias.view(BF)
        wpk[:, OFF_MASK:OFF_MASK + 2 * WM] = maskb.view(BF)
        return wpk

    ND, NS, CH = plan['ND'], plan['NS'], plan['CH']
    doff, soff, moff = plan['doff'], plan['soff'], plan['moff']
    cores = []
    for c in range(NCORES):
        dstT = np.zeros((H, plan['WD']), BF)
        srcT = np.zeros((H, plan['WS']), BF)
        maskb = np.full((128, WM), MASK_NEG, np.float32)
        for j in range(slots):
            g = plan['assign'][c, j]
            dstT[:, doff[j]:doff[j] + nd[g]] = dst_h[doff_g[g]:doff_g[g] + nd[g]].T.astype(BF)
            srcT[:, soff[j]:soff[j] + ns[g]] = src_h[soff_g[g]:soff_g[g] + ns[g]].T.astype(BF)
            for jj in range(CH[j]):
                valid = max(0, min(128, int(ns[g]) - jj * 128))
                maskb[:valid, moff[j] + jj] = 0.0
        cores.append(dict(dstT=dstT, srcT=srcT, wpack=pack_weights(maskb)))

    meta = dict(nd=nd, doff_g=doff_g, plan=plan)
    return cores, meta


def declare_tensors(nc, plan):
    WTOT = OFF_MASK + 2 * plan['WM']
    aps = {}
    aps['dstT'] = nc.dram_tensor("dstT", [H, plan['WD']], BF16, kind="ExternalInput").ap()
    aps['srcT'] = nc.dram_tensor("srcT", [H, plan['WS']], BF16, kind="ExternalInput").ap()
    aps['wpack'] = nc.dram_tensor("wpack", [128, WTOT], BF16, kind="ExternalInput").ap()
    aps['outT'] = nc.dram_tensor("outT", [H, plan['WD']], BF16, kind="ExternalOutput").ap()
    return aps


def _dram3(ap, col0, width, total_w):
    """AP over a [256, total_w] dram tensor: [p=row%128, a=row//128, w]."""
    return bass.AP(tensor=ap.tensor, offset=col0,
                   ap=[[total_w, 128], [128 * total_w, 2], [1, width]])


def build_body(ctx: ExitStack, tc: tile.TileContext, aps, plan):
    nc = tc.nc
    slots = plan['slots']
    ND, NS, CH = plan['ND'], plan['NS'], plan['CH']
    doff, soff, moff = plan['doff'], plan['soff'], plan['moff']
    NDmax = int(ND.max())
    NSmax = int(NS.max())
    CHmax = int(CH.max())
    WTOT = OFF_MASK + 2 * plan['WM']

    wp = ctx.enter_context(tc.tile_pool(name="wp", bufs=1))
    inp = ctx.enter_context(tc.tile_pool(name="inp", bufs=3))
    act = ctx.enter_context(tc.tile_pool(name="act", bufs=1))
    # PSUM banks: gpp(proj q/k + v-direct) 2 + sc 3 + msg 1 + mlp(den,y,z) 2 = 8
    gpp = ctx.enter_context(tc.tile_pool(name="gpp", bufs=2, space="PSUM"))
    scp = ctx.enter_context(tc.tile_pool(name="scp", bufs=3, space="PSUM"))
    mdp = ctx.enter_context(tc.tile_pool(name="mdp", bufs=1, space="PSUM"))
    mlp = ctx.enter_context(tc.tile_pool(name="mlp", bufs=2, space="PSUM"))

    # --- packed weights: one DMA; everything else is AP slices of it ---
    wt = wp.tile([128, WTOT], BF16, tag="wt")
    nc.scalar.dma_start(out=wt[:], in_=aps['wpack'][:])

    def wslice(off, width):
        return wt[:, off:off + width]

    def pbcol(i):
        return wt[:, OFF_PB + 2 * i:OFF_PB + 2 * i + 2].bitcast(F32)

    def maskcol(m):
        return wt[:, OFF_MASK + 2 * m:OFF_MASK + 2 * m + 2].bitcast(F32)

    # --- persistent V slots (zero-padded band layout), zero-filled once ---
    v_slots = []
    for i in range(3):
        vs = act.tile([128, CHmax, HEADS, C], BF16, tag=f"Vs{i}", name=f"Vs{i}")
        nc.vector.memset(vs[:], 0.0)
        v_slots.append(vs)

    # --- input tiles: 3-buffer rotation, 2-slot DMA lookahead ---
    dst_t = [None] * slots
    src_t = [None] * slots

    def load_slot(s):
        dt_ = inp.tile([128, 2, ND[s]], BF16, tag="dst", name=f"dst{s}")
        st_ = inp.tile([128, 2, NS[s]], BF16, tag="src", name=f"src{s}")
        nc.sync.dma_start(out=dt_[:], in_=_dram3(aps['dstT'], int(doff[s]), int(ND[s]), plan['WD']))
        nc.sync.dma_start(out=st_[:], in_=_dram3(aps['srcT'], int(soff[s]), int(NS[s]), plan['WS']))
        dst_t[s] = dt_
        src_t[s] = st_

    load_slot(0)
    if slots > 1:
        load_slot(1)

    for s in range(slots):
        nd_, ns_, ch = int(ND[s]), int(NS[s]), int(CH[s])
        if s + 2 < slots:
            load_slot(s + 2)
        dt_, st_ = dst_t[s], src_t[s]

        # ---------- projections ----------
        ps_q = gpp.tile([128, NDmax], F32, tag="gpp", name=f"psq{s}")
        for a in range(2):
            nc.tensor.matmul(ps_q[:, :nd_], wslice(OFF_WQ + a * 128, 128), dt_[:, a, :],
                             start=(a == 0), stop=(a == 1))
        q_t = act.tile([128, NDmax], BF16, tag="q", name=f"q{s}", bufs=2)
        nc.vector.tensor_scalar_add(q_t[:, :nd_], ps_q[:, :nd_], pbcol(0))

        ps_k = gpp.tile([128, NSmax], F32, tag="gpp", name=f"psk{s}")
        for a in range(2):
            nc.tensor.matmul(ps_k[:, :ns_], wslice(OFF_WK + a * 128, 128), st_[:, a, :],
                             start=(a == 0), stop=(a == 1))
        k_t = act.tile([128, NSmax], BF16, tag="k", name=f"k{s}", bufs=2)
        nc.vector.tensor_scalar_add(k_t[:, :ns_], ps_k[:, :ns_], pbcol(1))

        # ---------- V direct (natural [tok, chan]); bias folded into MLP1 ----------
        ps_vd = gpp.tile([128, CHmax, 128], F32, tag="gpp", name=f"psvd{s}")
        for j in range(ch):
            for a in range(2):
                nc.tensor.matmul(ps_vd[:, j, :],
                                 st_[:, a, j * 128:(j + 1) * 128],
                                 wslice(OFF_WV + a * 128, 128),
                                 start=(a == 0), stop=(a == 1))
        v_sb = v_slots[s % 3]
        vdst = bass.AP(tensor=v_sb.tensor, offset=v_sb.offset,
                       ap=[v_sb.ap[0]] + [[HEADS * C, ch], [C + DH, HEADS], [1, DH]])
        vsrc = bass.AP(tensor=ps_vd.tensor, offset=ps_vd.offset,
                       ap=[ps_vd.ap[0]] + [[128, ch], [DH, HEADS], [1, DH]])
        nc.vector.tensor_copy(vdst, vsrc)

        # ---------- attention ----------
        ps_msg = mdp.tile([128, NDmax], F32, tag="msg", name=f"psmsg{s}")
        e0 = [None] * HEADS     # head -> first chunk's e tile
        e_run = [None] * HEADS  # head -> chunk-sum accumulator
        for j in range(ch):
            for h in range(HEADS):
                ps_sc = scp.tile([128, 512], F32, tag="sc", name=f"pssc{s}_{j}_{h}")
                nc.tensor.matmul(
                    ps_sc[:, :nd_],
                    k_t[32 * h:32 * h + 32, j * 128:(j + 1) * 128],
                    q_t[32 * h:32 * h + 32, :nd_],
                    start=True, stop=True, tile_position=(32 * h, 0))
                e_t = act.tile([128, NDmax], BF16, tag="E", name=f"E{s}_{j}_{h}", bufs=6)
                nc.scalar.activation(e_t[:, :nd_], ps_sc[:, :nd_],
                                     mybir.ActivationFunctionType.Exp,
                                     bias=maskcol(moff[s] + j))
                nc.tensor.matmul(
                    ps_msg[:, :nd_],
                    v_sb[:, j, h, :],
                    e_t[:, :nd_],
                    start=(j == 0 and h == 0), stop=(j == ch - 1 and h == 3))
                # denominator: sum e over src chunks on DVE/GpSimd, one
                # matmul per head at the end
                eng = nc.vector if h < 2 else nc.gpsimd
                if ch == 1:
                    e_run[h] = e_t
                elif j == 0:
                    e0[h] = e_t
                elif j == 1:
                    er = act.tile([128, NDmax], BF16, tag=f"er{h}", name=f"er{s}_{h}", bufs=2)
                    eng.tensor_add(er[:, :nd_], e0[h][:, :nd_], e_t[:, :nd_])
                    e_run[h] = er
                else:
                    eng.tensor_add(e_run[h][:, :nd_], e_run[h][:, :nd_],
                                   e_t[:, :nd_])
        ps_den = mlp.tile([128, NDmax], F32, tag="mlp", name=f"psden{s}")
        for h in range(HEADS):
            nc.tensor.matmul(
                ps_den[:, :nd_],
                wslice(OFF_ONES + h * 128, 128),
                e_run[h][:, :nd_],
                start=(h == 0), stop=(h == 3))
        r_sb = act.tile([128, NDmax], F32, tag="rsb", name=f"rsb{s}", bufs=2)
        nc.vector.reciprocal_approx_fast(r_sb[:, :nd_], ps_den[:, :nd_])
        msgn = act.tile([128, NDmax], BF16, tag="msgn", name=f"msgn{s}", bufs=2)
        nc.vector.tensor_mul(msgn[:, :nd_], ps_msg[:, :nd_], r_sb[:, :nd_])

        # ---------- MLP (merge folded into W1; residual via eye matmul) ----------
        out_sb = act.tile([128, 2, NDmax], BF16, tag="out", name=f"out{s}", bufs=2)
        y1 = [None, None]
        for o in range(2):
            ps_y = mlp.tile([128, NDmax], F32, tag="mlp", name=f"psy{s}_{o}")
            rhs_list = [dt_[:, 0, :], dt_[:, 1, :], msgn[:, :nd_]]
            for kk in range(3):
                nc.tensor.matmul(ps_y[:, :nd_], wslice(OFF_W1 + kk * 256 + o * 128, 128),
                                 rhs_list[kk], start=(kk == 0), stop=(kk == 2))
            y1_t = act.tile([128, NDmax], BF16, tag=f"y1_{o}", name=f"y1_{s}_{o}", bufs=2)
            nc.vector.tensor_scalar(y1_t[:, :nd_], ps_y[:, :nd_], pbcol(2 + o), 0.0,
                                    op0=mybir.AluOpType.add, op1=mybir.AluOpType.max)
            y1[o] = y1_t
        for o in range(2):
            ps_z = mlp.tile([128, NDmax], F32, tag="mlp", name=f"psz{s}_{o}")
            for kk in range(2):
                nc.tensor.matmul(ps_z[:, :nd_], wslice(OFF_W2 + kk * 256 + o * 128, 128),
                                 y1[kk][:, :nd_], start=(kk == 0), stop=False)
            nc.tensor.matmul(ps_z[:, :nd_], wslice(OFF_EYE, 128), dt_[:, o, :],
                             start=False, stop=True)
            nc.vector.tensor_scalar_add(out_sb[:, o, :nd_], ps_z[:, :nd_], pbcol(4 + o))
        nc.sync.dma_start(out=_dram3(aps['outT'], int(doff[s]), nd_, plan['WD']),
                          in_=out_sb[:, :, :nd_])


def build_nc(plan=None):
    if plan is None:
        plan = _PLAN
    nc = bacc.Bacc("TRN2", target_bir_lowering=False, debug=False,
                   enable_asserts=True, num_devices=NCORES)
    aps = declare_tensors(nc, plan)
    with tile.TileContext(nc) as tc:
        with ExitStack() as ctx:
            build_body(ctx, tc, aps, plan)
    nc.compile()
    return nc


def in_map(core, shared=None):
    return dict(dstT=core['dstT'], srcT=core['srcT'], wpack=core['wpack'])


def assemble(outTs, meta):
    nd = meta['nd']
    doff_g = meta['doff_g']
    plan = meta['plan']
    out = np.empty((int(nd.sum()), H), np.float32)
    for c in range(NCORES):
        for j in range(plan['slots']):
            g = plan['assign'][c, j]
            sl = outTs[c][:, plan['doff'][j]: plan['doff'][j] + nd[g]]
            out[doff_g[g]:doff_g[g] + nd[g]] = sl.T.astype(np.float32)
    return out


def kernel(**inputs):
    cores, meta = host_prep(inputs)
    nc = build_nc(meta['plan'])
    in_maps = [in_map(cores[c]) for c in range(NCORES)]
    res = run_bass_kernel_spmd(nc, in_maps, core_ids=list(range(NCORES)))
    outTs = [np.asarray(res.results[c]["outT"]) for c in range(NCORES)]
    return assemble(outTs, meta)


# revision 14
# speedup vs baseline: 2.5811x; 1.0333x over previous
"""Trainium2 Bass kernel for nn_CrossAttentionLayer (ragged cross-attention + MLP).

Sharding: 64 ragged segments -> 8 cores x 8 slots. Segments are sorted by
(src-chunk count, dst count), dealt into slots of 8, then hill-climbed so
segments sharing a slot have similar sizes; each slot is trimmed to the max
dst count (ND) / src count (NS, rounded to 128) over its 8 segments, so all
cores run one SPMD program.

All matmul operands are bf16 (fp32 PSUM accumulation). Activations stay
channel-major [chan, tok]; softmax runs in scoresT orientation [src, dst]
with the src-padding mask applied as a per-partition bias on the exp. The
denominator is computed by summing e over src chunks on DVE/GpSimd, then one
banded-ones matmul per head replicates each head's denominator across its 32
partitions; normalization is reciprocal_approx_fast + multiply. V is
produced directly in natural [tok, chan] orientation (src chunks as
stationary); its bias and the merge conv + BN are folded into the MLP
weights on the host. The dst residual is added via an identity matmul
accumulated into the MLP2 PSUM group.

All weights/biases/masks ship in one packed bf16 DMA (f32 parts bitcast);
per-slot inputs/outputs move as single 3D-AP DMAs with a 3-buffer rotation
and 2-slot prefetch.
"""
import math
import sys
from contextlib import ExitStack

import numpy as np
import ml_dtypes

try:
    import concourse.bass as bass
except ImportError:
    sys.path.insert(0, "/opt/trn_rl_repo")
    import concourse.bass as bass

import concourse.tile as tile
from concourse import bacc, mybir
from concourse.bass_utils import run_bass_kernel_spmd

F32 = mybir.dt.float32
BF16 = mybir.dt.bfloat16
BF = ml_dtypes.bfloat16

H = 256          # h_dim
C = 128          # h_div
HEADS = 4
DH = 32
NCORES = 8
MASK_NEG = -20000.0

# packed-weights column offsets (bf16 elements per partition)
OFF_WQ = 0
OFF_WK = 256
OFF_WV = 512
OFF_W1 = 768
OFF_W2 = 1536
OFF_ONES = 2048
OFF_EYE = 2560
OFF_PB = 2688    # 6 f32 = 12 bf16
OFF_MASK = 2700  # WM f32 = 2*WM bf16

# Filled by host_prep; read by build_nc/in_map (same process).
_PLAN = {}


def _make_plan(nd, ns):
    """Assign 64 segments to (core, slot); compute per-slot widths."""
    B = len(nd)
    slots = B // NCORES
    chunks_of = np.ceil(ns / 128).astype(int)
    order = list(np.lexsort((-nd, -chunks_of)))

    def slot_cost(grp):
        ndm = -(-max(nd[g] for g in grp) // 4) * 4
        chm = max(chunks_of[g] for g in grp)
        return ndm * (14 + 8 * chm + 4) + 2 * 128 * chm + 256 * chm

    rng = np.random.default_rng(0)
    groups = [order[j * NCORES:(j + 1) * NCORES] for j in range(slots)]
    costs = [slot_cost(g) for g in groups]
    for _ in range(30000):
        j1, j2 = rng.integers(0, slots, 2)
        if j1 == j2:
            continue
        i1, i2 = rng.integers(0, NCORES, 2)
        g1, g2 = groups[j1][i1], groups[j2][i2]
        groups[j1][i1], groups[j2][i2] = g2, g1
        c1, c2 = slot_cost(groups[j1]), slot_cost(groups[j2])
        if c1 + c2 < costs[j1] + costs[j2]:
            costs[j1], costs[j2] = c1, c2
        else:
            groups[j1][i1], groups[j2][i2] = g1, g2
    # big slots first so the tail slot is the cheapest
    sidx = sorted(range(slots), key=lambda j: -costs[j])
    groups = [groups[j] for j in sidx]

    assign = np.empty((NCORES, slots), dtype=int)   # (core, slot) -> segment
    ND = np.empty(slots, dtype=int)
    NS = np.empty(slots, dtype=int)
    for j in range(slots):
        grp = groups[j]
        for c in range(NCORES):
            assign[c, j] = grp[c]
        ND[j] = int(-(-max(nd[g] for g in grp) // 4) * 4)       # mult of 4
        NS[j] = int(-(-max(ns[g] for g in grp) // 128) * 128)   # mult of 128
    CH = (NS // 128).astype(int)
    return dict(slots=slots, assign=assign, ND=ND, NS=NS, CH=CH,
                doff=np.concatenate([[0], np.cumsum(ND)[:-1]]),
                soff=np.concatenate([[0], np.cumsum(NS)[:-1]]),
                moff=np.concatenate([[0], np.cumsum(CH)[:-1]]),
                WD=int(ND.sum()), WS=int(NS.sum()), WM=int(CH.sum()))


def host_prep(inputs):
    src_h = np.asarray(inputs['src_h'], np.float32)
    dst_h = np.asarray(inputs['dst_h'], np.float32)
    ns = np.asarray(inputs['src_num_verts']).astype(np.int64)
    nd = np.asarray(inputs['dst_num_verts']).astype(np.int64)
    soff_g = np.concatenate([[0], np.cumsum(ns)[:-1]])
    doff_g = np.concatenate([[0], np.cumsum(nd)[:-1]])

    plan = _make_plan(nd, ns)
    global _PLAN
    _PLAN = plan
    slots = plan['slots']

    perm = np.empty(C, np.int64)
    for chat in range(C):
        h, d = divmod(chat, DH)
        perm[chat] = d * HEADS + h
    s = 1.0 / math.sqrt(DH)

    f32 = lambda k: np.asarray(inputs[k], np.float32)
    Wq, bq = f32('Wq'), f32('bq')
    Wk, bk = f32('Wk'), f32('bk')
    Wv, bv = f32('Wv'), f32('bv')
    Wm, bm = f32('Wm'), f32('bm')
    W1, b1 = f32('W1'), f32('b1')
    W2, b2 = f32('W2'), f32('b2')
    g1, be1, rm1, rv1 = f32('g1'), f32('be1'), f32('rm1'), f32('rv1')
    g2, be2, rm2, rv2 = f32('g2'), f32('be2'), f32('rm2'), f32('rv2')

    WqT = (Wq[perm] * s).T                                # [256,128]
    bq_s = bq[perm] * s
    WkT = Wk[perm].T
    WvT = Wv[perm].T
    bv_r = bv[perm]
    Wm_p = Wm[:, perm]
    a1 = g1 / np.sqrt(rv1 + 1e-5)
    W1_f = W1 * a1[:, None]
    b1_f = b1 * a1 + be1 - rm1 * a1
    a2 = g2 / np.sqrt(rv2 + 1e-5)
    W2_f = W2 * a2[:, None]
    b2_f = b2 * a2 + be2 - rm2 * a2
    W1m_p = W1_f[:, H:] @ Wm_p
    # V bias folded all the way into the MLP1 bias: msg enters MLP1 as
    # msg/den (no bias), and W1m_p @ bv_perm is a constant.
    b1_p = b1_f + W1_f[:, H:] @ bm + W1m_p @ bv_r
    W1T = np.concatenate([W1_f[:, :H], W1m_p], axis=1).T  # [384,256]
    W2T = W2_f.T                                          # [256,256]

    pbias = np.zeros((128, 6), np.float32)
    pbias[:, 0] = bq_s
    pbias[:, 1] = bk[perm]
    pbias[:, 2] = b1_p[:128]
    pbias[:, 3] = b1_p[128:]
    pbias[:, 4] = b2_f[:128]
    pbias[:, 5] = b2_f[128:]

    onespad = np.zeros((128, HEADS, C), BF)
    for h in range(HEADS):
        onespad[:, h, h * DH:(h + 1) * DH] = 1.0

    WM = plan['WM']
    WTOT = OFF_MASK + 2 * WM

    def pack_weights(maskb):
        wpk = np.zeros((128, WTOT), BF)
        wpk[:, OFF_WQ:OFF_WQ + 256] = WqT.reshape(2, 128, 128).transpose(1, 0, 2).reshape(128, 256).astype(BF)
        wpk[:, OFF_WK:OFF_WK + 256] = WkT.reshape(2, 128, 128).transpose(1, 0, 2).reshape(128, 256).astype(BF)
        wpk[:, OFF_WV:OFF_WV + 256] = WvT.reshape(2, 128, 128).transpose(1, 0, 2).reshape(128, 256).astype(BF)
        wpk[:, OFF_W1:OFF_W1 + 768] = W1T.reshape(3, 128, 256).transpose(1, 0, 2).reshape(128, 768).astype(BF)
        wpk[:, OFF_W2:OFF_W2 + 512] = W2T.reshape(2, 128, 256).transpose(1, 0, 2).reshape(128, 512).astype(BF)
        wpk[:, OFF_ONES:OFF_ONES + 512] = onespad.reshape(128, 512)
        wpk[:, OFF_EYE:OFF_EYE + 128] = np.eye(128, dtype=BF)
        wpk[:, OFF_PB:OFF_PB + 12] = pbias.view(BF)
        wpk[:, OFF_MASK:OFF_MASK + 2 * WM] = maskb.view(BF)
        return wpk

    ND, NS, CH = plan['ND'], plan['NS'], plan['CH']
    doff, soff, moff = plan['doff'], plan['soff'], plan['moff']
    cores = []
    for c in range(NCORES):
        dstT = np.zeros((H, plan['WD']), BF)
        srcT = np.zeros((H, plan['WS']), BF)
        maskb = np.full((128, WM), MASK_NEG, np.float32)
        for j in range(slots):
            g = plan['assign'][c, j]
            dstT[:, doff[j]:doff[j] + nd[g]] = dst_h[doff_g[g]:doff_g[g] + nd[g]].T.astype(BF)
            srcT[:, soff[j]:soff[j] + ns[g]] = src_h[soff_g[g]:soff_g[g] + ns[g]].T.astype(BF)
            for jj in range(CH[j]):
                valid = max(0, min(128, int(ns[g]) - jj * 128))
                maskb[:valid, moff[j] + jj] = 0.0
        cores.append(dict(dstT=dstT, srcT=srcT, wpack=pack_weights(maskb)))

    meta = dict(nd=nd, doff_g=doff_g, plan=plan)
    return cores, meta


def declare_tensors(nc, plan):
    WTOT = OFF_MASK + 2 * plan['WM']
    aps = {}
    aps['dstT'] = nc.dram_tensor("dstT", [H, plan['WD']], BF16, kind="ExternalInput").ap()
    aps['srcT'] = nc.dram_tensor("srcT", [H, plan['WS']], BF16, kind="ExternalInput").ap()
    aps['wpack'] = nc.dram_tensor("wpack", [128, WTOT], BF16, kind="ExternalInput").ap()
    aps['outT'] = nc.dram_tensor("outT", [H, plan['WD']], BF16, kind="ExternalOutput").ap()
    return aps


def _dram3(ap, col0, width, total_w):
    """AP over a [256, total_w] dram tensor: [p=row%128, a=row//128, w]."""
    return bass.AP(tensor=ap.tensor, offset=col0,
                   ap=[[total_w, 128], [128 * total_w, 2], [1, width]])


def build_body(ctx: ExitStack, tc: tile.TileContext, aps, plan):
    nc = tc.nc
    slots = plan['slots']
    ND, NS, CH = plan['ND'], plan['NS'], plan['CH']
    doff, soff, moff = plan['doff'], plan['soff'], plan['moff']
    NDmax = int(ND.max())
    NSmax = int(NS.max())
    CHmax = int(CH.max())
    WTOT = OFF_MASK + 2 * plan['WM']

    wp = ctx.enter_context(tc.tile_pool(name="wp", bufs=1))
    inp = ctx.enter_context(tc.tile_pool(name="inp", bufs=3))
    act = ctx.enter_context(tc.tile_pool(name="act", bufs=1))
    # PSUM banks: gpp(proj q/k + v-direct) 2 + sc 3 + msg 1 + mlp(den,y,z) 2 = 8
    gpp = ctx.enter_context(tc.tile_pool(name="gpp", bufs=2, space="PSUM"))
    scp = ctx.enter_context(tc.tile_pool(name="scp", bufs=3, space="PSUM"))
    mdp = ctx.enter_context(tc.tile_pool(name="mdp", bufs=1, space="PSUM"))
    mlp = ctx.enter_context(tc.tile_pool(name="mlp", bufs=2, space="PSUM"))

    # --- packed weights: one DMA; everything else is AP slices of it ---
    wt = wp.tile([128, WTOT], BF16, tag="wt")
    nc.scalar.dma_start(out=wt[:], in_=aps['wpack'][:])

    def wslice(off, width):
        return wt[:, off:off + width]

    def pbcol(i):
        return wt[:, OFF_PB + 2 * i:OFF_PB + 2 * i + 2].bitcast(F32)

    def maskcol(m):
        return wt[:, OFF_MASK + 2 * m:OFF_MASK + 2 * m + 2].bitcast(F32)

    # --- persistent V slots (zero-padded band layout), zero-filled once ---
    v_slots = []
    for i in range(3):
        vs = act.tile([128, CHmax, HEADS, C], BF16, tag=f"Vs{i}", name=f"Vs{i}")
        nc.gpsimd.memset(vs[:], 0.0)
        v_slots.append(vs)

    # --- input tiles: 3-buffer rotation, 2-slot DMA lookahead ---
    dst_t = [None] * slots
    src_t = [None] * slots

    def load_slot(s):
        dt_ = inp.tile([128, 2, ND[s]], BF16, tag="dst", name=f"dst{s}")
        st_ = inp.tile([128, 2, NS[s]], BF16, tag="src", name=f"src{s}")
        nc.sync.dma_start(out=dt_[:], in_=_dram3(aps['dstT'], int(doff[s]), int(ND[s]), plan['WD']))
        nc.sync.dma_start(out=st_[:], in_=_dram3(aps['srcT'], int(soff[s]), int(NS[s]), plan['WS']))
        dst_t[s] = dt_
        src_t[s] = st_

    load_slot(0)

    for s in range(slots):
        nd_, ns_, ch = int(ND[s]), int(NS[s]), int(CH[s])
        dt_, st_ = dst_t[s], src_t[s]

        # ---------- projections ----------
        ps_q = gpp.tile([128, NDmax], F32, tag="gpp", name=f"psq{s}")
        for a in range(2):
            nc.tensor.matmul(ps_q[:, :nd_], wslice(OFF_WQ + a * 128, 128), dt_[:, a, :],
                             start=(a == 0), stop=(a == 1))
        q_t = act.tile([128, NDmax], BF16, tag="q", name=f"q{s}", bufs=2)
        nc.vector.tensor_scalar_add(q_t[:, :nd_], ps_q[:, :nd_], pbcol(0))

        ps_k = gpp.tile([128, NSmax], F32, tag="gpp", name=f"psk{s}")
        for a in range(2):
            nc.tensor.matmul(ps_k[:, :ns_], wslice(OFF_WK + a * 128, 128), st_[:, a, :],
                             start=(a == 0), stop=(a == 1))
        k_t = act.tile([128, NSmax], BF16, tag="k", name=f"k{s}", bufs=2)
        nc.vector.tensor_scalar_add(k_t[:, :ns_], ps_k[:, :ns_], pbcol(1))

        # ---------- V direct (natural [tok, chan]); bias folded into MLP1 ----------
        ps_vd = gpp.tile([128, CHmax, 128], F32, tag="gpp", name=f"psvd{s}")
        for j in range(ch):
            for a in range(2):
                nc.tensor.matmul(ps_vd[:, j, :],
                                 st_[:, a, j * 128:(j + 1) * 128],
                                 wslice(OFF_WV + a * 128, 128),
                                 start=(a == 0), stop=(a == 1))
        v_sb = v_slots[s % 3]
        vdst = bass.AP(tensor=v_sb.tensor, offset=v_sb.offset,
                       ap=[v_sb.ap[0]] + [[HEADS * C, ch], [C + DH, HEADS], [1, DH]])
        vsrc = bass.AP(tensor=ps_vd.tensor, offset=ps_vd.offset,
                       ap=[ps_vd.ap[0]] + [[128, ch], [DH, HEADS], [1, DH]])
        nc.vector.tensor_copy(vdst, vsrc)

        if s + 1 < slots:
            load_slot(s + 1)

        # ---------- attention ----------
        ps_msg = mdp.tile([128, NDmax], F32, tag="msg", name=f"psmsg{s}")
        e0 = [None] * HEADS     # head -> first chunk's e tile
        e_run = [None] * HEADS  # head -> chunk-sum accumulator
        for j in range(ch):
            for h in range(HEADS):
                ps_sc = scp.tile([128, 512], F32, tag="sc", name=f"pssc{s}_{j}_{h}")
                nc.tensor.matmul(
                    ps_sc[:, :nd_],
                    k_t[32 * h:32 * h + 32, j * 128:(j + 1) * 128],
                    q_t[32 * h:32 * h + 32, :nd_],
                    start=True, stop=True, tile_position=(32 * h, 0))
                e_t = act.tile([128, NDmax], BF16, tag="E", name=f"E{s}_{j}_{h}", bufs=6)
                nc.scalar.activation(e_t[:, :nd_], ps_sc[:, :nd_],
                                     mybir.ActivationFunctionType.Exp,
                                     bias=maskcol(moff[s] + j))
                nc.tensor.matmul(
                    ps_msg[:, :nd_],
                    v_sb[:, j, h, :],
                    e_t[:, :nd_],
                    start=(j == 0 and h == 0), stop=(j == ch - 1 and h == 3))
                # denominator: sum e over src chunks on DVE/GpSimd, one
                # matmul per head at the end
                eng = nc.vector if h < 2 else nc.gpsimd
                if ch == 1:
                    e_run[h] = e_t
                elif j == 0:
                    e0[h] = e_t
                else:
                    er = act.tile([128, NDmax], BF16, tag=f"er{h}_{j % 2}",
                                  name=f"er{s}_{h}_{j}", bufs=2)
                    eng.tensor_add(er[:, :nd_], e_run[h][:, :nd_] if j > 1 else e0[h][:, :nd_],
                                   e_t[:, :nd_])
                    e_run[h] = er
        ps_den = mlp.tile([128, NDmax], F32, tag="mlp", name=f"psden{s}")
        for h in range(HEADS):
            nc.tensor.matmul(
                ps_den[:, :nd_],
                wslice(OFF_ONES + h * 128, 128),
                e_run[h][:, :nd_],
                start=(h == 0), stop=(h == 3))
        r_sb = act.tile([128, NDmax], F32, tag="rsb", name=f"rsb{s}", bufs=2)
        nc.vector.reciprocal_approx_fast(r_sb[:, :nd_], ps_den[:, :nd_])
        msgn = act.tile([128, NDmax], BF16, tag="msgn", name=f"msgn{s}", bufs=2)
        nc.vector.tensor_mul(msgn[:, :nd_], ps_msg[:, :nd_], r_sb[:, :nd_])

        # ---------- MLP (merge folded into W1; residual via eye matmul) ----------
        out_sb = act.tile([128, 2, NDmax], BF16, tag="out", name=f"out{s}", bufs=2)
        y1 = [None, None]
        for o in range(2):
            ps_y = mlp.tile([128, NDmax], F32, tag="mlp", name=f"psy{s}_{o}")
            rhs_list = [dt_[:, 0, :], dt_[:, 1, :], msgn[:, :nd_]]
            for kk in range(3):
                nc.tensor.matmul(ps_y[:, :nd_], wslice(OFF_W1 + kk * 256 + o * 128, 128),
                                 rhs_list[kk], start=(kk == 0), stop=(kk == 2))
            y1_t = act.tile([128, NDmax], BF16, tag=f"y1_{o}", name=f"y1_{s}_{o}", bufs=2)
            nc.vector.tensor_scalar(y1_t[:, :nd_], ps_y[:, :nd_], pbcol(2 + o), 0.0,
                                    op0=mybir.AluOpType.add, op1=mybir.AluOpType.max)
            y1[o] = y1_t
        for o in range(2):
            ps_z = mlp.tile([128, NDmax], F32, tag="mlp", name=f"psz{s}_{o}")
            for kk in range(2):
                nc.tensor.matmul(ps_z[:, :nd_], wslice(OFF_W2 + kk * 256 + o * 128, 128),
                                 y1[kk][:, :nd_], start=(kk == 0), stop=False)
            nc.tensor.matmul(ps_z[:, :nd_], wslice(OFF_EYE, 128), dt_[:, o, :],
                             start=False, stop=True)
            nc.vector.tensor_scalar_add(out_sb[:, o, :nd_], ps_z[:, :nd_], pbcol(4 + o))
        nc.sync.dma_start(out=_dram3(aps['outT'], int(doff[s]), nd_, plan['WD']),
                          in_=out_sb[:, :, :nd_])


def build_nc(plan=None):
    if plan is None:
        plan = _PLAN
    nc = bacc.Bacc("TRN2", target_bir_lowering=False, debug=False,
                   enable_asserts=True, num_devices=NCORES)
    aps = declare_tensors(nc, plan)
    with tile.TileContext(nc) as tc:
        with ExitStack() as ctx:
            build_body(ctx, tc, aps, plan)
    nc.compile()
    return nc


def in_map(core, shared=None):
    return dict(dstT=core['dstT'], srcT=core['srcT'], wpack=core['wpack'])


def assemble(outTs, meta):
    nd = meta['nd']
    doff_g = meta['doff_g']
    plan = meta['plan']
    out = np.empty((int(nd.sum()), H), np.float32)
    for c in range(NCORES):
        for j in range(plan['slots']):
            g = plan['assign'][c, j]
            sl = outTs[c][:, plan['doff'][j]: plan['doff'][j] + nd[g]]
            out[doff_g[g]:doff_g[g] + nd[g]] = sl.T.astype(np.float32)
    return out


def kernel(**inputs):
    cores, meta = host_prep(inputs)
    nc = build_nc(meta['plan'])
    in_maps = [in_map(cores[c]) for c in range(NCORES)]
    res = run_bass_kernel_spmd(nc, in_maps, core_ids=list(range(NCORES)))
    outTs = [np.asarray(res.results[c]["outT"]) for c in range(NCORES)]
    return assemble(outTs, meta)
